# revision 34
# baseline (speedup 1.0000x reference)
"""Trainium2 Bass kernel for Angles2Backbone (NeRF chain forward).

Full inputs: input [256,3,512] f32, param [6] f32, angles_length [256] i32.
Output: [256, 4608] f32  (coords of 1536 backbone atoms x 3, masked).

Sharding: pure data parallel over batch - 32 proteins per core x 8 cores.

Per-core algorithm (v2, residue-granularity scan):
  - Layout: 128 partitions = (quarter q in 0..3)*32 + protein b. Each row
    owns 128 consecutive residues (=384 atoms) of protein b's chain.
  - Pre-pass: per-residue product Rres = B_N @ B_CA @ B_C computed from
    cos/sin planes with per-type param scalars folded in (leaf structure
    collapses most terms into tensor_scalar/scalar_tensor_tensor ops).
  - Rotation prefix over 128 residues via Hillis-Steele (7 steps) on 9
    entry planes, ping-pong buffered, DVE/Pool split.
  - Cross-quarter fixup: gather quarter-end matrices, 3-step mini-scan,
    apply incoming prefix as per-partition scalars.
  - Atom translations: u_a = R_a * (first column of atom-level prefix)
    expands from residue prefixes with precomputed v-vectors; per-row
    prefix sum via hardware tensor_tensor_scan; additive cross-quarter
    fixup + length mask fused into the final store.
"""

import sys

sys.path.insert(0, "/opt/trn_rl_repo")

import numpy as np
import concourse.bass as bass
import concourse.bacc as bacc
import concourse.mybir as mybir
from concourse import tile
from concourse.bass_utils import run_bass_kernel_spmd

F32 = mybir.dt.float32
I32 = mybir.dt.int32
AF = mybir.ActivationFunctionType
OP = mybir.AluOpType

NCORES = 8
BPC = 32          # proteins per core
L = 512           # residues per protein
QN = 4            # chain quarters per protein (partition groups)
W = 384           # atoms per quarter
NR = 128          # residues per quarter (scan length)
PI = float(np.pi)

_CACHE = {}


def _e(i, k):
    return 3 * i + k


def _build_graph():
    nc = bacc.Bacc("TRN2", target_bir_lowering=False, debug=False,
                   num_devices=NCORES)
    inp = nc.dram_tensor("input", [QN * BPC, 3 * NR], F32,
                     kind="ExternalInput").ap()
    par = nc.dram_tensor("param", [6], F32, kind="ExternalInput").ap()
    alen = nc.dram_tensor("angles_length", [BPC], I32,
                          kind="ExternalInput").ap()
    out = nc.dram_tensor("out", [QN * BPC, 3 * W], F32,
                     kind="ExternalOutput").ap()

    with tile.TileContext(nc) as tc:
        _emit(nc, tc, inp, par, alen, out)
    nc.compile()
    return nc


def _emit(nc, tc, inp, par, alen, out):
    import contextlib
    ctx = contextlib.ExitStack()
    with ctx:
        main = ctx.enter_context(tc.tile_pool(name="main", bufs=1))
        tmps = ctx.enter_context(tc.tile_pool(name="tmps", bufs=18))

        # ---------------- persistent tiles ----------------
        alpha = main.tile([128, W], F32, tag="alpha")
        ca = main.tile([128, W], F32, tag="ca")
        sa = main.tile([128, W], F32, tag="sa")
        C1 = main.tile([128, 9 * NR], F32, tag="C1")
        RA = main.tile([128, 9 * NR], F32, tag="RA")   # residue mats / Pfull
        RB = main.tile([128, 9 * NR], F32, tag="RB")   # ping-pong
        PP = main.tile([128, 4 * NR], F32, tag="PP")   # pp1..pp4
        QQ = main.tile([128, 6 * NR], F32, tag="QQ")   # q1_i, q2_i
        Vm = main.tile([128, 9 * NR], F32, tag="Vm")   # v1,v2,v3 x 3 coords
        zeros = main.tile([128, W], F32, tag="zeros")
        ones = main.tile([128, NR], F32, tag="ones")
        Pall = main.tile([128, 3 * W], F32, tag="Pall")
        Pmall = main.tile([128, 3 * W], F32, tag="Pmall")
        jplane_i = main.tile([128, W], I32, tag="jplane_i")
        jplane = main.tile([128, W], F32, tag="jplane")
        maskp = main.tile([128, W], F32, tag="maskp")
        thr = main.tile([128, 1], F32, tag="thr")
        Lbc = main.tile([128, 1], F32, tag="Lbc")
        Lsb = main.tile([BPC, 1], I32, tag="Lsb")
        Lf = main.tile([BPC, 1], F32, tag="Lf")
        Psb = main.tile([1, 6], F32, tag="Psb")
        kv = main.tile([1, 3], F32, tag="kv")
        Rv = main.tile([1, 3], F32, tag="Rv")
        NSC = 24
        vecs = main.tile([1, NSC], F32, tag="vecs")
        Vb = main.tile([128, NSC], F32, tag="Vb")
        Estack = main.tile([BPC, 36], F32, tag="Estack")
        Fstack = main.tile([BPC, 27], F32, tag="Fstack")
        Fbc = main.tile([128, 9], F32, tag="Fbc")
        pestage = main.tile([BPC, 9], F32, tag="pestage")
        cumst = main.tile([BPC, 9], F32, tag="cumst")
        Pincb = main.tile([128, 3], F32, tag="Pincb")
        zb1 = main.tile([1, 1], F32, tag="zb1")
        zb128 = main.tile([128, 1], F32, tag="zb128")

        _cnt = [0]

        def ENG():
            # TT ops only: alternate DVE (2/3) and Pool (1/3)
            _cnt[0] += 1
            return nc.gpsimd if (_cnt[0] % 3 == 0) else nc.vector

        # ---------------- input DMA: assemble alpha ----------------
        # inp viewed as [c][q][b][m] so one DMA covers all 4 quarters
        # (dst partition order is (q, b), matching the AP walk order).
        av = alpha[:]
        nc.scalar.dma_start(Psb[:], par[:])
        nc.scalar.dma_start(Lsb[:], alen[:])
        nc.sync.dma_start(av[:, :], inp[:])

        # ---------------- param scalars ----------------
        for t, idx in enumerate((5, 1, 3)):   # kappa: CA_C_N, C_N_CA, N_CA_C
            nc.vector.tensor_copy(kv[0:1, t:t + 1], Psb[0:1, idx:idx + 1])
        for t, idx in enumerate((4, 0, 2)):   # R: R_C_N, R_N_CA, R_CA_C
            nc.vector.tensor_copy(Rv[0:1, t:t + 1], Psb[0:1, idx:idx + 1])
        nc.vector.memset(zb1[:], 0.0)
        nc.vector.memset(zb128[:], 0.0)
        # per-type ck/sk: sk=sin(kappa) (kappa in (0,pi)); ck=1-2sin^2(k/2)
        sk3 = main.tile([1, 3], F32, tag="sk3")
        ck3 = main.tile([1, 3], F32, tag="ck3")
        kvr = main.tile([1, 3], F32, tag="kvr")
        nc.scalar.activation(sk3[:], kv[0:1, 0:3], AF.Sin, bias=zb1[:])
        nc.scalar.activation(kvr[:], kv[0:1, 0:3], AF.Sin, bias=zb1[:],
                             scale=0.5)
        nc.scalar.square(kvr[:], kvr[:])
        nc.vector.tensor_scalar(ck3[:], kvr[:], -2.0, 1.0,
                                op0=OP.mult, op1=OP.add)

        # scalar slot layout in vecs[1, NSC]:
        # 0:ckN 1:skN 2:ckA 3:skA 4:ckC 5:skC
        # 6:ckNckA 7:skNskA 8:ckNskA 9:skNckA
        # 10:nskNckA 11:nckNskA 12:nckN 13:nckA 14:nckC 15:nskA
        # 16:RNckN 17:RNskN 18:RCA 19:RC
        def vc(i):
            return vecs[0:1, i:i + 1]

        for t in range(3):
            nc.vector.tensor_copy(vc(2 * t), ck3[0:1, t:t + 1])
            nc.vector.tensor_copy(vc(2 * t + 1), sk3[0:1, t:t + 1])
        nc.vector.tensor_mul(vc(6), vc(0), vc(2))     # ckN*ckA
        nc.vector.tensor_mul(vc(7), vc(1), vc(3))     # skN*skA
        nc.vector.tensor_mul(vc(8), vc(0), vc(3))     # ckN*skA
        nc.vector.tensor_mul(vc(9), vc(1), vc(2))     # skN*ckA
        nc.vector.tensor_scalar_mul(vc(10), vc(9), -1.0)
        nc.vector.tensor_scalar_mul(vc(11), vc(8), -1.0)
        nc.vector.tensor_scalar_mul(vc(12), vc(0), -1.0)
        nc.vector.tensor_scalar_mul(vc(13), vc(2), -1.0)
        nc.vector.tensor_scalar_mul(vc(14), vc(4), -1.0)
        nc.vector.tensor_scalar_mul(vc(15), vc(3), -1.0)
        nc.vector.tensor_mul(vc(16), Rv[0:1, 0:1], vc(0))   # RN*ckN
        nc.vector.tensor_mul(vc(17), Rv[0:1, 0:1], vc(1))   # RN*skN
        nc.vector.tensor_copy(vc(18), Rv[0:1, 1:2])         # R_CA
        nc.vector.tensor_copy(vc(19), Rv[0:1, 2:3])         # R_C
        nc.gpsimd.partition_broadcast(Vb[:], vecs[:])

        S = {}
        for i, nm in enumerate(("ckN", "skN", "ckA", "skA", "ckC", "skC",
                                "ckNckA", "skNskA", "ckNskA", "skNckA",
                                "nskNckA", "nckNskA", "nckN", "nckA",
                                "nckC", "nskA", "RNckN", "RNskN",
                                "RCA", "RC")):
            S[nm] = Vb[:, i:i + 1]

        # trig: |alpha| < 4pi: s4=sin(a/4), c4=1-2sin^2(a/8);
        # s2=2*s4*c4, c2=1-2*s4^2; s1=2*s2*c2, c1=1-2*s2^2.
        # One chain per 128-col type block, pipelined across ACT/DVE.
        for t in range(3):
            bs = slice(t * NR, (t + 1) * NR)
            avb, cab, sab = av[:, bs], ca[:, bs], sa[:, bs]
            ts8 = tmps.tile([128, NR], F32, tag="t1")
            ts4 = tmps.tile([128, NR], F32, tag="t2")
            tq = tmps.tile([128, NR], F32, tag="t1")
            nc.scalar.activation(ts8[:], avb, AF.Sin, bias=zb128[:],
                                 scale=0.125)
            nc.scalar.activation(ts4[:], avb, AF.Sin, bias=zb128[:],
                                 scale=0.25)
            nc.scalar.square(ts8[:], ts8[:])
            nc.vector.tensor_scalar(cab, ts8[:], -2.0, 1.0,
                                    op0=OP.mult, op1=OP.add)          # c4
            nc.vector.scalar_tensor_tensor(ts8[:], ts4[:], 2.0, cab,
                                           op0=OP.mult, op1=OP.mult)  # s2
            nc.scalar.square(tq[:], ts4[:])
            nc.vector.tensor_scalar(ts4[:], tq[:], -2.0, 1.0,
                                    op0=OP.mult, op1=OP.add)          # c2
            nc.vector.scalar_tensor_tensor(sab, ts8[:], 2.0, ts4[:],
                                           op0=OP.mult, op1=OP.mult)  # s1
            nc.scalar.square(tq[:], ts8[:])
            nc.vector.tensor_scalar(cab, tq[:], -2.0, 1.0,
                                    op0=OP.mult, op1=OP.add)          # c1
        nc.gpsimd.memset(zeros[:], 0.0)
        nc.gpsimd.memset(zeros[:], 0.0)
        nc.gpsimd.memset(ones[:], 1.0)

        # per-type strided cos/sin views [128, 128]
        cN, sN = ca[:, 0:128], sa[:, 0:128]
        cA, sA = ca[:, 128:256], sa[:, 128:256]
        cC, sC = ca[:, 256:384], sa[:, 256:384]

        def blk(t, e, lo=0, hi=NR):
            return t[:, e * NR + lo:e * NR + hi]

        V = nc.vector
        STT = nc.vector.scalar_tensor_tensor
        TS = nc.vector.tensor_scalar

        # ---------------- pre-pass: C1 = B_N @ B_CA ----------------
        pp1 = PP[:, 0 * NR:1 * NR]
        pp2 = PP[:, 1 * NR:2 * NR]
        pp3 = PP[:, 2 * NR:3 * NR]
        pp4 = PP[:, 3 * NR:4 * NR]
        nc.gpsimd.tensor_mul(pp1, cN, cA)
        nc.gpsimd.tensor_mul(pp2, sN, sA)
        nc.gpsimd.tensor_mul(pp3, cN, sA)
        nc.gpsimd.tensor_mul(pp4, sN, cA)
        c1 = C1[:]
        TS(blk(c1, 0), cA, S["skNskA"], S["ckNckA"],
           op0=OP.mult, op1=OP.add)                       # C1_00
        TS(blk(c1, 1), cA, S["nskNckA"], S["ckNskA"],
           op0=OP.mult, op1=OP.add)                       # C1_01
        V.tensor_scalar_mul(blk(c1, 2), sA, S["skN"])     # C1_02
        x = blk(c1, 3)                                    # C1_10
        nc.scalar.mul(x, cN, S["skNckA"])
        STT(x, pp1, S["nckNskA"], x, op0=OP.mult, op1=OP.add)
        STT(x, pp2, S["skA"], x, op0=OP.mult, op1=OP.add)
        x = blk(c1, 4)                                    # C1_11
        nc.scalar.mul(x, cN, S["skNskA"])
        STT(x, pp1, S["ckNckA"], x, op0=OP.mult, op1=OP.add)
        STT(x, pp2, S["nckA"], x, op0=OP.mult, op1=OP.add)
        STT(blk(c1, 5), pp3, S["nckN"], pp4,
            op0=OP.mult, op1=OP.subtract)                 # C1_12
        x = blk(c1, 6)                                    # C1_20
        nc.scalar.mul(x, sN, S["skNckA"])
        STT(x, pp4, S["nckNskA"], x, op0=OP.mult, op1=OP.add)
        STT(x, pp3, S["nskA"], x, op0=OP.mult, op1=OP.add)
        x = blk(c1, 7)                                    # C1_21
        nc.scalar.mul(x, sN, S["skNskA"])
        STT(x, pp4, S["ckNckA"], x, op0=OP.mult, op1=OP.add)
        STT(x, pp3, S["ckA"], x, op0=OP.mult, op1=OP.add)
        STT(blk(c1, 8), pp2, S["nckN"], pp1,
            op0=OP.mult, op1=OP.add)                      # C1_22

        # residue-0 of q=0: B_N := Identity => C1 := B_CA(0)
        # (alpha_CA(0)=0 so cA=1, sA=0 there): [[ckA,skA,0],[skA,-ckA,0],
        # [0,0,-1]]
        r0s = slice(0, BPC)
        o1 = ones[r0s, 0:1]
        V.tensor_scalar_mul(c1[r0s, 0 * NR:0 * NR + 1], o1, S["ckA"][r0s])
        V.tensor_scalar_mul(c1[r0s, 1 * NR:1 * NR + 1], o1, S["skA"][r0s])
        V.memset(c1[r0s, 2 * NR:2 * NR + 1], 0.0)
        V.tensor_scalar_mul(c1[r0s, 3 * NR:3 * NR + 1], o1, S["skA"][r0s])
        V.tensor_scalar_mul(c1[r0s, 4 * NR:4 * NR + 1], o1, S["nckA"][r0s])
        V.memset(c1[r0s, 5 * NR:5 * NR + 1], 0.0)
        V.memset(c1[r0s, 6 * NR:6 * NR + 1], 0.0)
        V.memset(c1[r0s, 7 * NR:7 * NR + 1], 0.0)
        V.memset(c1[r0s, 8 * NR:8 * NR + 1], -1.0)

        # ---------------- pre-pass: Rres = C1 @ B_C -> RA ----------------
        ra = RA[:]
        for i in range(3):
            nc.gpsimd.tensor_mul(blk(QQ[:], i), blk(c1, _e(i, 1)), cC)
            nc.gpsimd.tensor_mul(blk(QQ[:], 3 + i), blk(c1, _e(i, 2)), sC)
        for i in range(3):
            q1i = blk(QQ[:], i)
            q2i = blk(QQ[:], 3 + i)
            x = blk(ra, _e(i, 0))
            nc.scalar.mul(x, blk(c1, _e(i, 0)), S["ckC"])
            STT(x, q1i, S["skC"], x, op0=OP.mult, op1=OP.add)
            STT(x, q2i, S["skC"], x, op0=OP.mult, op1=OP.add)
            x = blk(ra, _e(i, 1))
            nc.scalar.mul(x, blk(c1, _e(i, 0)), S["skC"])
            STT(x, q1i, S["nckC"], x, op0=OP.mult, op1=OP.add)
            STT(x, q2i, S["nckC"], x, op0=OP.mult, op1=OP.add)
            t1 = tmps.tile([128, NR], F32, tag="pt1")
            t2 = tmps.tile([128, NR], F32, tag="pt2")
            nc.gpsimd.tensor_mul(t1[:], blk(c1, _e(i, 1)), sC)
            nc.gpsimd.tensor_mul(t2[:], blk(c1, _e(i, 2)), cC)
            nc.gpsimd.tensor_sub(blk(ra, _e(i, 2)), t1[:], t2[:])

        # ---------------- v-vectors for atom expansion ----------------
        # v1 = t_N = RN*(ckN, skN*cN, skN*sN); v2 = RCA*C1[:,0];
        # v3 = RC*Rres[:,0]
        vm = Vm[:]
        nc.scalar.mul(blk(vm, 0), ones[:], S["RNckN"])
        nc.scalar.mul(blk(vm, 1), cN, S["RNskN"])
        nc.scalar.mul(blk(vm, 2), sN, S["RNskN"])
        for i in range(3):
            nc.scalar.mul(blk(vm, 3 + i), blk(c1, _e(i, 0)), S["RCA"])
            nc.scalar.mul(blk(vm, 6 + i), blk(ra, _e(i, 0)), S["RC"])

        # ---------------- Hillis-Steele residue scan ----------------
        # Fused step: all 9 output entries in one 3-dim AP op per k-term:
        #   out[i,j] += L[i,k] (bcast over j) * R[k,j] (bcast over i)
        # 5 logical ops per step, each split col-wise DVE/Pool.
        def ap3(base_ap, off, dims):
            return bass.AP(base_ap.tensor, base_ap.offset + off,
                           [list(base_ap.ap[0])] + [list(d) for d in dims])

        T9a = main.tile([128, 9 * NR], F32, tag="T9a")
        T9b = main.tile([128, 9 * NR], F32, tag="T9b")

        def fused_step(srcb, dstb, s):
            n = NR - s
            cut = (n * 24) // 35        # DVE share of columns
            sv = srcb.rearrange("p (e j) -> p e j", e=9)
            dv = dstb.rearrange("p (e j) -> p e j", e=9)
            nc.scalar.copy(dv[:, :, 0:s], sv[:, :, 0:s])

            def L(k, c0, c1):
                return ap3(srcb, k * NR + c0,
                           [[3 * NR, 3], [0, 3], [1, c1 - c0]])

            def R(k, c0, c1):
                return ap3(srcb, 3 * k * NR + s + c0,
                           [[0, 3], [NR, 3], [1, c1 - c0]])

            def T(t, c0, c1):
                return ap3(t[:], c0, [[3 * NR, 3], [NR, 3], [1, c1 - c0]])

            def O(c0, c1):
                return ap3(dstb, s + c0, [[3 * NR, 3], [NR, 3], [1, c1 - c0]])

            for E, c0, c1 in ((nc.vector, 0, cut), (nc.gpsimd, cut, n)):
                if c1 <= c0:
                    continue
                E.tensor_mul(T(T9a, c0, c1), L(0, c0, c1), R(0, c0, c1))
                E.tensor_mul(T(T9b, c0, c1), L(1, c0, c1), R(1, c0, c1))
                E.tensor_add(T(T9a, c0, c1), T(T9a, c0, c1), T(T9b, c0, c1))
                E.tensor_mul(T(T9b, c0, c1), L(2, c0, c1), R(2, c0, c1))
                E.tensor_add(O(c0, c1), T(T9a, c0, c1), T(T9b, c0, c1))

        bufs = [RA, RB]
        nsteps = 7
        for step in range(nsteps):
            fused_step(bufs[step % 2][:], bufs[(step + 1) % 2][:],
                       1 << step)
        Rscan = bufs[nsteps % 2][:]    # RB: local residue prefixes

        # ---------------- cross-quarter rotation fixup ----------------
        for q in range(QN):
            (nc.sync if q % 2 else nc.scalar).dma_start(
                Estack[0:BPC, q * 9:(q + 1) * 9],
                Rscan[q * BPC:(q + 1) * BPC, NR - 1:9 * NR:NR])
        nc.vector.tensor_copy(Fstack[0:BPC, 0:9], Estack[0:BPC, 0:9])
        mt0 = main.tile([BPC, 9], F32, tag="mt0")
        mt1 = main.tile([BPC, 9], F32, tag="mt1")
        fs = Fstack[:]
        es = Estack[:]

        def ap2(base_ap, off, dims):
            return bass.AP(base_ap.tensor, base_ap.offset + off,
                           [list(base_ap.ap[0])] + [list(d) for d in dims])

        for q in (1, 2):
            FL = lambda k: ap2(fs, (q - 1) * 9 + k, [[3, 3], [0, 3]])
            ER = lambda k: ap2(es, q * 9 + 3 * k, [[0, 3], [1, 3]])
            MT = lambda t: ap2(t[:], 0, [[3, 3], [1, 3]])
            FO = ap2(fs, q * 9, [[3, 3], [1, 3]])
            V.tensor_mul(MT(mt0), FL(0), ER(0))
            V.tensor_mul(MT(mt1), FL(1), ER(1))
            V.tensor_add(MT(mt0), MT(mt0), MT(mt1))
            V.tensor_mul(MT(mt1), FL(2), ER(2))
            V.tensor_add(FO, MT(mt0), MT(mt1))
        nc.vector.memset(Fbc[0:BPC, 0:9], 0.0)
        for e in (0, 4, 8):
            nc.vector.memset(Fbc[0:BPC, e:e + 1], 1.0)
        for q in (1, 2, 3):
            (nc.sync if q % 2 else nc.scalar).dma_start(
                Fbc[q * BPC:(q + 1) * BPC, 0:9],
                              Fstack[0:BPC, (q - 1) * 9:q * 9])
        # ---------------- atom translations (local frame) ----------------
        # u_local[c-plane, atom 3r+m] = sum_k Plocal_{r-1}[c,k] * v_m[k]
        # fused over (c, m) via broadcast APs, col-split DVE/Pool.
        Uloc = main.tile([128, 3 * W], F32, tag="Uloc")
        ul = Uloc[:]
        V.tensor_copy(ap3(ul, 0, [[W, 3], [1, 3]]),
                      ap3(vm, 0, [[NR, 3], [3 * NR, 3]]))
        nu = NR - 1
        ucut = (nu * 24) // 35

        def UL(k, c0, c1):
            return ap3(Rscan, k * NR + c0,
                       [[3 * NR, 3], [0, 3], [1, c1 - c0]])

        def UR(k, c0, c1):
            return ap3(vm, k * NR + 1 + c0,
                       [[0, 3], [3 * NR, 3], [1, c1 - c0]])

        def UT(t, c0, c1):
            return ap3(t[:], c0, [[3 * NR, 3], [NR, 3], [1, c1 - c0]])

        def UO(c0, c1):
            return ap3(ul, 3 + 3 * c0, [[W, 3], [1, 3], [3, c1 - c0]])

        for E, c0, c1 in ((nc.vector, 0, ucut), (nc.gpsimd, ucut, nu)):
            E.tensor_mul(UT(T9a, c0, c1), UL(0, c0, c1), UR(0, c0, c1))
            E.tensor_mul(UT(T9b, c0, c1), UL(1, c0, c1), UR(1, c0, c1))
            E.tensor_add(UT(T9a, c0, c1), UT(T9a, c0, c1), UT(T9b, c0, c1))
            E.tensor_mul(UT(T9b, c0, c1), UL(2, c0, c1), UR(2, c0, c1))
            E.tensor_add(UO(c0, c1), UT(T9a, c0, c1), UT(T9b, c0, c1))
        # prefix-sum the LOCAL u per coordinate (frame fix applied at the
        # end by linearity: sum_j F@u = F@sum_j u)
        for c in range(3):
            uc = ul[:, c * W:(c + 1) * W]
            V.memset(uc[0:BPC, 0:1], 0.0)   # atom 0 of the whole chain
            nc.vector.tensor_tensor_scan(
                Pall[:, c * W:(c + 1) * W], uc, zeros[:], 0.0,
                op0=OP.add, op1=OP.add)

        # ---------------- cross-quarter translation fixup ----------------
        pv = Pall[:]
        for q in range(3):
            (nc.sync if q % 2 else nc.scalar).dma_start(
                pestage[0:BPC, q * 3:(q + 1) * 3],
                              pv[q * BPC:(q + 1) * BPC, W - 1:3 * W:W])
        # global pe_q = F_q @ pe_local_q (F_0 = I); Fstack block q-1 = F_q
        peg = main.tile([BPC, 9], F32, tag="peg")
        ps = pestage[:]
        nc.vector.tensor_copy(peg[0:BPC, 0:3], pestage[0:BPC, 0:3])
        for q in (1, 2):
            FL = lambda k: ap2(fs, (q - 1) * 9 + k, [[3, 3]])
            PR = lambda k: ap2(ps, q * 3 + k, [[0, 3]])
            M3 = lambda t: ap2(t[:], 0, [[1, 3]])
            PO = ap2(peg[:], q * 3, [[1, 3]])
            V.tensor_mul(M3(mt0), FL(0), PR(0))
            V.tensor_mul(M3(mt1), FL(1), PR(1))
            V.tensor_add(M3(mt0), M3(mt0), M3(mt1))
            V.tensor_mul(M3(mt1), FL(2), PR(2))
            V.tensor_add(PO, M3(mt0), M3(mt1))
        nc.vector.tensor_copy(cumst[0:BPC, 0:3], peg[0:BPC, 0:3])
        nc.vector.tensor_add(cumst[0:BPC, 3:6], cumst[0:BPC, 0:3],
                             peg[0:BPC, 3:6])
        nc.vector.tensor_add(cumst[0:BPC, 6:9], cumst[0:BPC, 3:6],
                             peg[0:BPC, 6:9])
        nc.vector.memset(Pincb[0:BPC, 0:3], 0.0)
        for q in (1, 2, 3):
            (nc.sync if q % 2 else nc.scalar).dma_start(
                Pincb[q * BPC:(q + 1) * BPC, 0:3],
                              cumst[0:BPC, (q - 1) * 3:q * 3])

        # ---------------- mask ----------------
        nc.gpsimd.iota(jplane_i[:], [[1, W]], channel_multiplier=0)
        nc.vector.tensor_copy(jplane[:], jplane_i[:])
        nc.vector.tensor_copy(Lf[:], Lsb[:])
        for q in range(QN):
            (nc.sync if q % 2 else nc.scalar).dma_start(
                Lbc[q * BPC:(q + 1) * BPC, 0:1], Lf[:])
        for q in range(QN):
            TS(thr[q * BPC:(q + 1) * BPC, 0:1],
               Lbc[q * BPC:(q + 1) * BPC, 0:1],
               3.0, float(q * W), op0=OP.mult, op1=OP.subtract)
        TS(maskp[:], jplane[:], thr[:, 0:1], None, op0=OP.is_lt)

        # ------------- fused frame-fix + P_inc + mask + store -------------
        for c in range(3):
            x = tmps.tile([128, W], F32, tag="t1")
            V.tensor_scalar_mul(x[:], pv[:, 0:W],
                                Fbc[:, _e(c, 0):_e(c, 0) + 1])
            STT(x[:], pv[:, W:2 * W], Fbc[:, _e(c, 1):_e(c, 1) + 1], x[:],
                op0=OP.mult, op1=OP.add)
            STT(x[:], pv[:, 2 * W:3 * W], Fbc[:, _e(c, 2):_e(c, 2) + 1], x[:],
                op0=OP.mult, op1=OP.add)
            STT(Pmall[:, c * W:(c + 1) * W], x[:],
                Pincb[:, c:c + 1], maskp[:], op0=OP.add, op1=OP.mult)
        nc.sync.dma_start(out[:], Pmall[:])


def _prep_alpha(input):
    # pure indexing: alphaN[r]=psi[r-1], alphaCA[r]=omega[r-1] (0 at r=0),
    # alphaC[r]=phi[r]; blocked (q, b, type, m).
    phi, psi, om = input[:, 0], input[:, 1], input[:, 2]
    z1 = np.zeros((input.shape[0], 1), np.float32)
    aN = np.concatenate([z1, psi[:, :-1]], axis=1)
    aCA = np.concatenate([z1, om[:, :-1]], axis=1)
    alpha = np.stack([aN, aCA, phi], axis=1)          # [B, 3, 512]
    return alpha.reshape(-1, 3, QN, NR).transpose(0, 2, 1, 3)


def _shard_alpha(alpha, i):
    sl = slice(i * BPC, (i + 1) * BPC)
    return np.ascontiguousarray(
        alpha[sl].transpose(1, 0, 2, 3).reshape(QN * BPC, 3 * NR))


def _get_nc():
    if "nc" not in _CACHE:
        _CACHE["nc"] = _build_graph()
    return _CACHE["nc"]


def kernel(input, param, angles_length, trace=False):
    input = np.ascontiguousarray(input, dtype=np.float32)
    param = np.ascontiguousarray(param, dtype=np.float32)
    angles_length = np.ascontiguousarray(angles_length, dtype=np.int32)
    nc = _get_nc()
    alpha = _prep_alpha(input)
    in_maps = []
    for i in range(NCORES):
        sl = slice(i * BPC, (i + 1) * BPC)
        in_maps.append({
            "input": _shard_alpha(alpha, i),
            "param": param,
            "angles_length": angles_length[sl],
        })
    res = run_bass_kernel_spmd(nc, in_maps, core_ids=list(range(NCORES)),
                               trace=trace)
    outs = []
    for i in range(NCORES):
        r = res.results[i]["out"]          # [(q,b), (c,j)]
        r = r.reshape(QN, BPC, 3, W)
        r = np.transpose(r, (1, 0, 3, 2)).reshape(BPC, 3 * QN * W)
        outs.append(r)
    full = np.concatenate(outs, axis=0).astype(np.float32)
    if trace:
        kernel._last_exec_ns = res.exec_time_ns
    return full


kernel._last_exec_ns = None


# revision 39
# speedup vs baseline: 1.1134x; 1.1134x over previous
"""Trainium2 Bass kernel for Angles2Backbone (NeRF chain forward).

Full inputs: input [256,3,512] f32, param [6] f32, angles_length [256] i32.
Output: [256, 4608] f32  (coords of 1536 backbone atoms x 3, masked).

Sharding: pure data parallel over batch - 32 proteins per core x 8 cores.

Per-core algorithm (v2, residue-granularity scan):
  - Layout: 128 partitions = (quarter q in 0..3)*32 + protein b. Each row
    owns 128 consecutive residues (=384 atoms) of protein b's chain.
  - Pre-pass: per-residue product Rres = B_N @ B_CA @ B_C computed from
    cos/sin planes with per-type param scalars folded in (leaf structure
    collapses most terms into tensor_scalar/scalar_tensor_tensor ops).
  - Rotation prefix over 128 residues via Hillis-Steele (7 steps) on 9
    entry planes, ping-pong buffered, DVE/Pool split.
  - Cross-quarter fixup: gather quarter-end matrices, 3-step mini-scan,
    apply incoming prefix as per-partition scalars.
  - Atom translations: u_a = R_a * (first column of atom-level prefix)
    expands from residue prefixes with precomputed v-vectors; per-row
    prefix sum via hardware tensor_tensor_scan; additive cross-quarter
    fixup + length mask fused into the final store.
"""

import sys

sys.path.insert(0, "/opt/trn_rl_repo")

import numpy as np
import concourse.bass as bass
import concourse.bacc as bacc
import concourse.mybir as mybir
from concourse import tile
from concourse.bass_utils import run_bass_kernel_spmd

F32 = mybir.dt.float32
I32 = mybir.dt.int32
AF = mybir.ActivationFunctionType
OP = mybir.AluOpType

NCORES = 8
BPC = 32          # proteins per core
L = 512           # residues per protein
QN = 4            # chain quarters per protein (partition groups)
W = 384           # atoms per quarter
NR = 128          # residues per quarter (scan length)
PI = float(np.pi)

_CACHE = {}


def _e(i, k):
    return 3 * i + k


def _build_graph():
    nc = bacc.Bacc("TRN2", target_bir_lowering=False, debug=False,
                   num_devices=NCORES)
    inp = nc.dram_tensor("input", [QN * BPC, 3 * NR], F32,
                     kind="ExternalInput").ap()
    par = nc.dram_tensor("param", [6], F32, kind="ExternalInput").ap()
    alen = nc.dram_tensor("angles_length", [BPC], I32,
                          kind="ExternalInput").ap()
    out = nc.dram_tensor("out", [QN * BPC, 3 * W], F32,
                     kind="ExternalOutput").ap()

    with tile.TileContext(nc) as tc:
        _emit(nc, tc, inp, par, alen, out)
    nc.compile()
    return nc


def _emit(nc, tc, inp, par, alen, out):
    import contextlib
    ctx = contextlib.ExitStack()
    with ctx:
        main = ctx.enter_context(tc.tile_pool(name="main", bufs=1))
        tmps = ctx.enter_context(tc.tile_pool(name="tmps", bufs=18))

        # ---------------- persistent tiles ----------------
        alpha = main.tile([128, W], F32, tag="alpha")
        ca = main.tile([128, W], F32, tag="ca")
        sa = main.tile([128, W], F32, tag="sa")
        C1 = main.tile([128, 9 * NR], F32, tag="C1")
        RA = main.tile([128, 9 * NR], F32, tag="RA")   # residue mats / Pfull
        RB = main.tile([128, 9 * NR], F32, tag="RB")   # ping-pong
        PP = main.tile([128, 4 * NR], F32, tag="PP")   # pp1..pp4
        QQ = main.tile([128, 6 * NR], F32, tag="QQ")   # q1_i, q2_i
        Vm = main.tile([128, 9 * NR], F32, tag="Vm")   # v1,v2,v3 x 3 coords
        zeros = main.tile([128, W], F32, tag="zeros")
        ones = main.tile([128, NR], F32, tag="ones")
        Pall = main.tile([128, 3 * W], F32, tag="Pall")
        Pmall = main.tile([128, 3 * W], F32, tag="Pmall")
        jplane_i = main.tile([128, W], I32, tag="jplane_i")
        jplane = main.tile([128, W], F32, tag="jplane")
        maskp = main.tile([128, W], F32, tag="maskp")
        thr = main.tile([128, 1], F32, tag="thr")
        Lbc = main.tile([128, 1], F32, tag="Lbc")
        Lsb = main.tile([BPC, 1], I32, tag="Lsb")
        Lf = main.tile([BPC, 1], F32, tag="Lf")
        Psb = main.tile([1, 6], F32, tag="Psb")
        kv = main.tile([1, 3], F32, tag="kv")
        Rv = main.tile([1, 3], F32, tag="Rv")
        NSC = 24
        vecs = main.tile([1, NSC], F32, tag="vecs")
        Vb = main.tile([128, NSC], F32, tag="Vb")
        Estack = main.tile([BPC, 36], F32, tag="Estack")
        Fstack = main.tile([BPC, 27], F32, tag="Fstack")
        Fbc = main.tile([128, 9], F32, tag="Fbc")
        pestage = main.tile([BPC, 9], F32, tag="pestage")
        cumst = main.tile([BPC, 9], F32, tag="cumst")
        Pincb = main.tile([128, 3], F32, tag="Pincb")
        zb1 = main.tile([1, 1], F32, tag="zb1")
        zb128 = main.tile([128, 1], F32, tag="zb128")

        _cnt = [0]

        def ENG():
            # TT ops only: alternate DVE (2/3) and Pool (1/3)
            _cnt[0] += 1
            return nc.gpsimd if (_cnt[0] % 3 == 0) else nc.vector

        # ---------------- input DMA: assemble alpha ----------------
        # inp viewed as [c][q][b][m] so one DMA covers all 4 quarters
        # (dst partition order is (q, b), matching the AP walk order).
        av = alpha[:]
        nc.scalar.dma_start(Psb[:], par[:])
        nc.scalar.dma_start(Lsb[:], alen[:])
        nc.sync.dma_start(av[:, :], inp[:])

        # ---------------- param scalars ----------------
        for t, idx in enumerate((5, 1, 3)):   # kappa: CA_C_N, C_N_CA, N_CA_C
            nc.vector.tensor_copy(kv[0:1, t:t + 1], Psb[0:1, idx:idx + 1])
        for t, idx in enumerate((4, 0, 2)):   # R: R_C_N, R_N_CA, R_CA_C
            nc.vector.tensor_copy(Rv[0:1, t:t + 1], Psb[0:1, idx:idx + 1])
        nc.vector.memset(zb1[:], 0.0)
        nc.vector.memset(zb128[:], 0.0)
        # per-type ck/sk: sk=sin(kappa) (kappa in (0,pi)); ck=1-2sin^2(k/2)
        sk3 = main.tile([1, 3], F32, tag="sk3")
        ck3 = main.tile([1, 3], F32, tag="ck3")
        kvr = main.tile([1, 3], F32, tag="kvr")
        nc.scalar.activation(sk3[:], kv[0:1, 0:3], AF.Sin, bias=zb1[:])
        nc.scalar.activation(kvr[:], kv[0:1, 0:3], AF.Sin, bias=zb1[:],
                             scale=0.5)
        nc.scalar.square(kvr[:], kvr[:])
        nc.vector.tensor_scalar(ck3[:], kvr[:], -2.0, 1.0,
                                op0=OP.mult, op1=OP.add)

        # scalar slot layout in vecs[1, NSC]:
        # 0:ckN 1:skN 2:ckA 3:skA 4:ckC 5:skC
        # 6:ckNckA 7:skNskA 8:ckNskA 9:skNckA
        # 10:nskNckA 11:nckNskA 12:nckN 13:nckA 14:nckC 15:nskA
        # 16:RNckN 17:RNskN 18:RCA 19:RC
        def vc(i):
            return vecs[0:1, i:i + 1]

        for t in range(3):
            nc.vector.tensor_copy(vc(2 * t), ck3[0:1, t:t + 1])
            nc.vector.tensor_copy(vc(2 * t + 1), sk3[0:1, t:t + 1])
        nc.vector.tensor_mul(vc(6), vc(0), vc(2))     # ckN*ckA
        nc.vector.tensor_mul(vc(7), vc(1), vc(3))     # skN*skA
        nc.vector.tensor_mul(vc(8), vc(0), vc(3))     # ckN*skA
        nc.vector.tensor_mul(vc(9), vc(1), vc(2))     # skN*ckA
        nc.vector.tensor_scalar_mul(vc(10), vc(9), -1.0)
        nc.vector.tensor_scalar_mul(vc(11), vc(8), -1.0)
        nc.vector.tensor_scalar_mul(vc(12), vc(0), -1.0)
        nc.vector.tensor_scalar_mul(vc(13), vc(2), -1.0)
        nc.vector.tensor_scalar_mul(vc(14), vc(4), -1.0)
        nc.vector.tensor_scalar_mul(vc(15), vc(3), -1.0)
        nc.vector.tensor_mul(vc(16), Rv[0:1, 0:1], vc(0))   # RN*ckN
        nc.vector.tensor_mul(vc(17), Rv[0:1, 0:1], vc(1))   # RN*skN
        nc.vector.tensor_copy(vc(18), Rv[0:1, 1:2])         # R_CA
        nc.vector.tensor_copy(vc(19), Rv[0:1, 2:3])         # R_C
        nc.gpsimd.partition_broadcast(Vb[:], vecs[:])

        S = {}
        for i, nm in enumerate(("ckN", "skN", "ckA", "skA", "ckC", "skC",
                                "ckNckA", "skNskA", "ckNskA", "skNckA",
                                "nskNckA", "nckNskA", "nckN", "nckA",
                                "nckC", "nskA", "RNckN", "RNskN",
                                "RCA", "RC")):
            S[nm] = Vb[:, i:i + 1]

        # trig: |alpha| < 4pi: s4=sin(a/4), c4=1-2sin^2(a/8);
        # s2=2*s4*c4, c2=1-2*s4^2; s1=2*s2*c2, c1=1-2*s2^2.
        # One chain per 128-col type block, pipelined across ACT/DVE.
        for t in range(3):
            bs = slice(t * NR, (t + 1) * NR)
            avb, cab, sab = av[:, bs], ca[:, bs], sa[:, bs]
            ts8 = tmps.tile([128, NR], F32, tag="t1")
            ts4 = tmps.tile([128, NR], F32, tag="t2")
            tq = tmps.tile([128, NR], F32, tag="t1")
            nc.scalar.activation(ts8[:], avb, AF.Sin, bias=zb128[:],
                                 scale=0.125)
            nc.scalar.activation(ts4[:], avb, AF.Sin, bias=zb128[:],
                                 scale=0.25)
            nc.scalar.square(ts8[:], ts8[:])
            nc.vector.tensor_scalar(cab, ts8[:], -2.0, 1.0,
                                    op0=OP.mult, op1=OP.add)          # c4
            nc.vector.scalar_tensor_tensor(ts8[:], ts4[:], 2.0, cab,
                                           op0=OP.mult, op1=OP.mult)  # s2
            nc.scalar.square(tq[:], ts4[:])
            nc.vector.tensor_scalar(ts4[:], tq[:], -2.0, 1.0,
                                    op0=OP.mult, op1=OP.add)          # c2
            nc.vector.scalar_tensor_tensor(sab, ts8[:], 2.0, ts4[:],
                                           op0=OP.mult, op1=OP.mult)  # s1
            nc.scalar.square(tq[:], ts8[:])
            nc.vector.tensor_scalar(cab, tq[:], -2.0, 1.0,
                                    op0=OP.mult, op1=OP.add)          # c1
        nc.gpsimd.memset(zeros[:], 0.0)
        nc.gpsimd.memset(zeros[:], 0.0)
        nc.gpsimd.memset(ones[:], 1.0)

        # per-type strided cos/sin views [128, 128]
        cN, sN = ca[:, 0:128], sa[:, 0:128]
        cA, sA = ca[:, 128:256], sa[:, 128:256]
        cC, sC = ca[:, 256:384], sa[:, 256:384]

        def blk(t, e, lo=0, hi=NR):
            return t[:, e * NR + lo:e * NR + hi]

        V = nc.vector
        STT = nc.vector.scalar_tensor_tensor
        TS = nc.vector.tensor_scalar

        # ---------------- pre-pass: C1 = B_N @ B_CA ----------------
        pp1 = PP[:, 0 * NR:1 * NR]
        pp2 = PP[:, 1 * NR:2 * NR]
        pp3 = PP[:, 2 * NR:3 * NR]
        pp4 = PP[:, 3 * NR:4 * NR]
        nc.gpsimd.tensor_mul(pp1, cN, cA)
        nc.gpsimd.tensor_mul(pp2, sN, sA)
        nc.gpsimd.tensor_mul(pp3, cN, sA)
        nc.gpsimd.tensor_mul(pp4, sN, cA)
        c1 = C1[:]
        TS(blk(c1, 0), cA, S["skNskA"], S["ckNckA"],
           op0=OP.mult, op1=OP.add)                       # C1_00
        TS(blk(c1, 1), cA, S["nskNckA"], S["ckNskA"],
           op0=OP.mult, op1=OP.add)                       # C1_01
        V.tensor_scalar_mul(blk(c1, 2), sA, S["skN"])     # C1_02
        x = blk(c1, 3)                                    # C1_10
        nc.scalar.mul(x, cN, S["skNckA"])
        STT(x, pp1, S["nckNskA"], x, op0=OP.mult, op1=OP.add)
        STT(x, pp2, S["skA"], x, op0=OP.mult, op1=OP.add)
        x = blk(c1, 4)                                    # C1_11
        nc.scalar.mul(x, cN, S["skNskA"])
        STT(x, pp1, S["ckNckA"], x, op0=OP.mult, op1=OP.add)
        STT(x, pp2, S["nckA"], x, op0=OP.mult, op1=OP.add)
        STT(blk(c1, 5), pp3, S["nckN"], pp4,
            op0=OP.mult, op1=OP.subtract)                 # C1_12
        x = blk(c1, 6)                                    # C1_20
        nc.scalar.mul(x, sN, S["skNckA"])
        STT(x, pp4, S["nckNskA"], x, op0=OP.mult, op1=OP.add)
        STT(x, pp3, S["nskA"], x, op0=OP.mult, op1=OP.add)
        x = blk(c1, 7)                                    # C1_21
        nc.scalar.mul(x, sN, S["skNskA"])
        STT(x, pp4, S["ckNckA"], x, op0=OP.mult, op1=OP.add)
        STT(x, pp3, S["ckA"], x, op0=OP.mult, op1=OP.add)
        STT(blk(c1, 8), pp2, S["nckN"], pp1,
            op0=OP.mult, op1=OP.add)                      # C1_22

        # residue-0 of q=0: B_N := Identity => C1 := B_CA(0)
        # (alpha_CA(0)=0 so cA=1, sA=0 there): [[ckA,skA,0],[skA,-ckA,0],
        # [0,0,-1]]
        r0s = slice(0, BPC)
        o1 = ones[r0s, 0:1]
        V.tensor_scalar_mul(c1[r0s, 0 * NR:0 * NR + 1], o1, S["ckA"][r0s])
        V.tensor_scalar_mul(c1[r0s, 1 * NR:1 * NR + 1], o1, S["skA"][r0s])
        V.memset(c1[r0s, 2 * NR:2 * NR + 1], 0.0)
        V.tensor_scalar_mul(c1[r0s, 3 * NR:3 * NR + 1], o1, S["skA"][r0s])
        V.tensor_scalar_mul(c1[r0s, 4 * NR:4 * NR + 1], o1, S["nckA"][r0s])
        V.memset(c1[r0s, 5 * NR:5 * NR + 1], 0.0)
        V.memset(c1[r0s, 6 * NR:6 * NR + 1], 0.0)
        V.memset(c1[r0s, 7 * NR:7 * NR + 1], 0.0)
        V.memset(c1[r0s, 8 * NR:8 * NR + 1], -1.0)

        # ---------------- pre-pass: Rres = C1 @ B_C -> RA ----------------
        ra = RA[:]
        for i in range(3):
            nc.gpsimd.tensor_mul(blk(QQ[:], i), blk(c1, _e(i, 1)), cC)
            nc.gpsimd.tensor_mul(blk(QQ[:], 3 + i), blk(c1, _e(i, 2)), sC)
        for i in range(3):
            q1i = blk(QQ[:], i)
            q2i = blk(QQ[:], 3 + i)
            x = blk(ra, _e(i, 0))
            nc.scalar.mul(x, blk(c1, _e(i, 0)), S["ckC"])
            STT(x, q1i, S["skC"], x, op0=OP.mult, op1=OP.add)
            STT(x, q2i, S["skC"], x, op0=OP.mult, op1=OP.add)
            x = blk(ra, _e(i, 1))
            nc.scalar.mul(x, blk(c1, _e(i, 0)), S["skC"])
            STT(x, q1i, S["nckC"], x, op0=OP.mult, op1=OP.add)
            STT(x, q2i, S["nckC"], x, op0=OP.mult, op1=OP.add)
            t1 = tmps.tile([128, NR], F32, tag="pt1")
            t2 = tmps.tile([128, NR], F32, tag="pt2")
            nc.gpsimd.tensor_mul(t1[:], blk(c1, _e(i, 1)), sC)
            nc.gpsimd.tensor_mul(t2[:], blk(c1, _e(i, 2)), cC)
            nc.gpsimd.tensor_sub(blk(ra, _e(i, 2)), t1[:], t2[:])

        # ---------------- v-vectors for atom expansion ----------------
        # v1 = t_N = RN*(ckN, skN*cN, skN*sN); v2 = RCA*C1[:,0];
        # v3 = RC*Rres[:,0]
        vm = Vm[:]
        nc.scalar.mul(blk(vm, 0), ones[:], S["RNckN"])
        nc.scalar.mul(blk(vm, 1), cN, S["RNskN"])
        nc.scalar.mul(blk(vm, 2), sN, S["RNskN"])
        for i in range(3):
            nc.scalar.mul(blk(vm, 3 + i), blk(c1, _e(i, 0)), S["RCA"])
            nc.scalar.mul(blk(vm, 6 + i), blk(ra, _e(i, 0)), S["RC"])

        # ---------------- Hillis-Steele residue scan ----------------
        # Fused step: all 9 output entries in one 3-dim AP op per k-term:
        #   out[i,j] += L[i,k] (bcast over j) * R[k,j] (bcast over i)
        # 5 logical ops per step, each split col-wise DVE/Pool.
        def ap3(base_ap, off, dims):
            return bass.AP(base_ap.tensor, base_ap.offset + off,
                           [list(base_ap.ap[0])] + [list(d) for d in dims])

        T9a = main.tile([128, 9 * NR], F32, tag="T9a")
        T9b = main.tile([128, 9 * NR], F32, tag="T9b")

        def fused_step(srcb, dstb, s, nr):
            n = nr - s
            cut = (n * 5) // 8          # DVE share of columns
            sv = srcb.rearrange("p (e j) -> p e j", e=9)
            dv = dstb.rearrange("p (e j) -> p e j", e=9)
            nc.scalar.copy(dv[:, :, 0:s], sv[:, :, 0:s])

            def L(k, c0, c1):
                return ap3(srcb, k * nr + c0,
                           [[3 * nr, 3], [0, 3], [1, c1 - c0]])

            def R(k, c0, c1):
                return ap3(srcb, 3 * k * nr + s + c0,
                           [[0, 3], [nr, 3], [1, c1 - c0]])

            def T(t, c0, c1):
                return ap3(t[:], c0, [[3 * nr, 3], [nr, 3], [1, c1 - c0]])

            def O(c0, c1):
                return ap3(dstb, s + c0, [[3 * nr, 3], [nr, 3], [1, c1 - c0]])

            for E, c0, c1 in ((nc.vector, 0, cut), (nc.gpsimd, cut, n)):
                if c1 <= c0:
                    continue
                E.tensor_mul(T(T9a, c0, c1), L(0, c0, c1), R(0, c0, c1))
                E.tensor_mul(T(T9b, c0, c1), L(1, c0, c1), R(1, c0, c1))
                E.tensor_add(T(T9a, c0, c1), T(T9a, c0, c1), T(T9b, c0, c1))
                E.tensor_mul(T(T9b, c0, c1), L(2, c0, c1), R(2, c0, c1))
                E.tensor_add(O(c0, c1), T(T9a, c0, c1), T(T9b, c0, c1))

        # pair adjacent residues: P2[r'] = Rres[2r'] @ Rres[2r'+1]
        NR2 = NR // 2
        P2A = main.tile([128, 9 * NR2], F32, tag="P2A")
        P2B = main.tile([128, 9 * NR2], F32, tag="P2B")
        pcut = (NR2 * 5) // 8
        ra_ap = RA[:]

        def PL(k, c0, c1):
            return ap3(ra_ap, k * NR + 2 * c0,
                       [[3 * NR, 3], [0, 3], [2, c1 - c0]])

        def PR(k, c0, c1):
            return ap3(ra_ap, 3 * k * NR + 1 + 2 * c0,
                       [[0, 3], [NR, 3], [2, c1 - c0]])

        def PT(t, c0, c1):
            return ap3(t[:], c0, [[3 * NR2, 3], [NR2, 3], [1, c1 - c0]])

        def PO(c0, c1):
            return ap3(P2A[:], c0, [[3 * NR2, 3], [NR2, 3], [1, c1 - c0]])

        for E, c0, c1 in ((nc.vector, 0, pcut), (nc.gpsimd, pcut, NR2)):
            E.tensor_mul(PT(T9a, c0, c1), PL(0, c0, c1), PR(0, c0, c1))
            E.tensor_mul(PT(T9b, c0, c1), PL(1, c0, c1), PR(1, c0, c1))
            E.tensor_add(PT(T9a, c0, c1), PT(T9a, c0, c1), PT(T9b, c0, c1))
            E.tensor_mul(PT(T9b, c0, c1), PL(2, c0, c1), PR(2, c0, c1))
            E.tensor_add(PO(c0, c1), PT(T9a, c0, c1), PT(T9b, c0, c1))

        Wodd = main.tile([128, 9 * NR2], F32, tag="Wodd")
        T9c = main.tile([128, 9 * NR2], F32, tag="T9c")
        T9d = main.tile([128, 9 * NR2], F32, tag="T9d")
        ul = Uloc[:]
        wo = Wodd[:]
        rs = Rscan
        wcut = (NR2 * 5) // 8

        def WL(k, c0, c1):
            return ap3(ra_ap, k * NR + 2 * c0,
                       [[3 * NR, 3], [0, 3], [2, c1 - c0]])

        def WR(k, c0, c1):
            return ap3(vm, k * NR + 1 + 2 * c0,
                       [[0, 3], [3 * NR, 3], [2, c1 - c0]])

        def WT(t, c0, c1):
            return ap3(t[:], c0, [[3 * NR2, 3], [NR2, 3], [1, c1 - c0]])

        def WO(c0, c1):
            return ap3(wo, c0, [[NR2, 3], [3 * NR2, 3], [1, c1 - c0]])

        for E, c0, c1 in ((nc.vector, 0, wcut), (nc.gpsimd, wcut, NR2)):
            E.tensor_mul(WT(T9a, c0, c1), WL(0, c0, c1), WR(0, c0, c1))
            E.tensor_mul(WT(T9b, c0, c1), WL(1, c0, c1), WR(1, c0, c1))
            E.tensor_add(WT(T9a, c0, c1), WT(T9a, c0, c1), WT(T9b, c0, c1))
            E.tensor_mul(WT(T9b, c0, c1), WL(2, c0, c1), WR(2, c0, c1))
            E.tensor_add(WO(c0, c1), WT(T9a, c0, c1), WT(T9b, c0, c1))


        bufs = [P2A, P2B]
        nsteps = 6
        for step in range(nsteps):
            fused_step(bufs[step % 2][:], bufs[(step + 1) % 2][:],
                       1 << step, NR2)
        Rscan = bufs[nsteps % 2][:]    # RB: local residue prefixes

        # ---------------- cross-quarter rotation fixup ----------------
        for q in range(QN):
            (nc.sync if q % 2 else nc.scalar).dma_start(
                Estack[0:BPC, q * 9:(q + 1) * 9],
                Rscan[q * BPC:(q + 1) * BPC, NR2 - 1:9 * NR2:NR2])
        nc.vector.tensor_copy(Fstack[0:BPC, 0:9], Estack[0:BPC, 0:9])
        mt0 = main.tile([BPC, 9], F32, tag="mt0")
        mt1 = main.tile([BPC, 9], F32, tag="mt1")
        fs = Fstack[:]
        es = Estack[:]

        def ap2(base_ap, off, dims):
            return bass.AP(base_ap.tensor, base_ap.offset + off,
                           [list(base_ap.ap[0])] + [list(d) for d in dims])

        for q in (1, 2):
            FL = lambda k: ap2(fs, (q - 1) * 9 + k, [[3, 3], [0, 3]])
            ER = lambda k: ap2(es, q * 9 + 3 * k, [[0, 3], [1, 3]])
            MT = lambda t: ap2(t[:], 0, [[3, 3], [1, 3]])
            FO = ap2(fs, q * 9, [[3, 3], [1, 3]])
            V.tensor_mul(MT(mt0), FL(0), ER(0))
            V.tensor_mul(MT(mt1), FL(1), ER(1))
            V.tensor_add(MT(mt0), MT(mt0), MT(mt1))
            V.tensor_mul(MT(mt1), FL(2), ER(2))
            V.tensor_add(FO, MT(mt0), MT(mt1))
        nc.vector.memset(Fbc[0:BPC, 0:9], 0.0)
        for e in (0, 4, 8):
            nc.vector.memset(Fbc[0:BPC, e:e + 1], 1.0)
        for q in (1, 2, 3):
            (nc.sync if q % 2 else nc.scalar).dma_start(
                Fbc[q * BPC:(q + 1) * BPC, 0:9],
                              Fstack[0:BPC, (q - 1) * 9:q * 9])
        # ---------------- atom translations (local frame) ----------------
        # superblock = 2 residues = 6 atoms. w_m (m=0..5): prefix-within-
        # superblock applied to t-vectors; m<3 are the per-residue v's at
        # even residues, m>=3 need Rres_even @ v_odd (fused below).
        Uloc = main.tile([128, 3 * W], F32, tag="Uloc")
        # superblock 0: local prefix = identity -> u = w_m
        V.tensor_copy(ap3(ul, 0, [[W, 3], [1, 3]]),
                      ap3(vm, 0, [[NR, 3], [3 * NR, 3]]))
        V.tensor_copy(ap3(ul, 3, [[W, 3], [1, 3]]),
                      ap3(wo, 0, [[NR2, 3], [3 * NR2, 3]]))
        nu = NR2 - 1
        ucut = (nu * 5) // 8

        def UL(k, c0, c1):
            return ap3(rs, k * NR2 + c0, [[3 * NR2, 3], [0, 3], [1, c1 - c0]])

        def UREV(k, c0, c1):   # even residues 2r', r'=1..63
            return ap3(vm, k * NR + 2 + 2 * c0,
                       [[0, 3], [3 * NR, 3], [2, c1 - c0]])

        def UROD(k, c0, c1):   # w_{3..5} at superblocks 1..63
            return ap3(wo, k * NR2 + 1 + c0,
                       [[0, 3], [3 * NR2, 3], [1, c1 - c0]])

        def UT(t, c0, c1):
            return ap3(t[:], c0, [[3 * NR2, 3], [NR2, 3], [1, c1 - c0]])

        def UO(off, c0, c1):
            return ap3(ul, off + 6 * c0, [[W, 3], [1, 3], [6, c1 - c0]])

        for E, c0, c1 in ((nc.vector, 0, ucut), (nc.gpsimd, ucut, nu)):
            E.tensor_mul(UT(T9c, c0, c1), UL(0, c0, c1), UREV(0, c0, c1))
            E.tensor_mul(UT(T9d, c0, c1), UL(1, c0, c1), UREV(1, c0, c1))
            E.tensor_add(UT(T9c, c0, c1), UT(T9c, c0, c1), UT(T9d, c0, c1))
            E.tensor_mul(UT(T9d, c0, c1), UL(2, c0, c1), UREV(2, c0, c1))
            E.tensor_add(UO(6, c0, c1), UT(T9c, c0, c1), UT(T9d, c0, c1))
        for E, c0, c1 in ((nc.vector, 0, ucut), (nc.gpsimd, ucut, nu)):
            E.tensor_mul(UT(T9a, c0, c1), UL(0, c0, c1), UROD(0, c0, c1))
            E.tensor_mul(UT(T9b, c0, c1), UL(1, c0, c1), UROD(1, c0, c1))
            E.tensor_add(UT(T9a, c0, c1), UT(T9a, c0, c1), UT(T9b, c0, c1))
            E.tensor_mul(UT(T9b, c0, c1), UL(2, c0, c1), UROD(2, c0, c1))
            E.tensor_add(UO(9, c0, c1), UT(T9a, c0, c1), UT(T9b, c0, c1))
        # prefix-sum the LOCAL u per coordinate (frame fix applied at the
        # end by linearity: sum_j F@u = F@sum_j u)
        for c in range(3):
            uc = ul[:, c * W:(c + 1) * W]
            V.memset(uc[0:BPC, 0:1], 0.0)   # atom 0 of the whole chain
            nc.vector.tensor_tensor_scan(
                Pall[:, c * W:(c + 1) * W], uc, zeros[:], 0.0,
                op0=OP.add, op1=OP.add)

        # ---------------- cross-quarter translation fixup ----------------
        pv = Pall[:]
        for q in range(3):
            (nc.sync if q % 2 else nc.scalar).dma_start(
                pestage[0:BPC, q * 3:(q + 1) * 3],
                              pv[q * BPC:(q + 1) * BPC, W - 1:3 * W:W])
        # global pe_q = F_q @ pe_local_q (F_0 = I); Fstack block q-1 = F_q
        peg = main.tile([BPC, 9], F32, tag="peg")
        ps = pestage[:]
        nc.vector.tensor_copy(peg[0:BPC, 0:3], pestage[0:BPC, 0:3])
        for q in (1, 2):
            FL = lambda k: ap2(fs, (q - 1) * 9 + k, [[3, 3]])
            PR = lambda k: ap2(ps, q * 3 + k, [[0, 3]])
            M3 = lambda t: ap2(t[:], 0, [[1, 3]])
            PO = ap2(peg[:], q * 3, [[1, 3]])
            V.tensor_mul(M3(mt0), FL(0), PR(0))
            V.tensor_mul(M3(mt1), FL(1), PR(1))
            V.tensor_add(M3(mt0), M3(mt0), M3(mt1))
            V.tensor_mul(M3(mt1), FL(2), PR(2))
            V.tensor_add(PO, M3(mt0), M3(mt1))
        nc.vector.tensor_copy(cumst[0:BPC, 0:3], peg[0:BPC, 0:3])
        nc.vector.tensor_add(cumst[0:BPC, 3:6], cumst[0:BPC, 0:3],
                             peg[0:BPC, 3:6])
        nc.vector.tensor_add(cumst[0:BPC, 6:9], cumst[0:BPC, 3:6],
                             peg[0:BPC, 6:9])
        nc.vector.memset(Pincb[0:BPC, 0:3], 0.0)
        for q in (1, 2, 3):
            (nc.sync if q % 2 else nc.scalar).dma_start(
                Pincb[q * BPC:(q + 1) * BPC, 0:3],
                              cumst[0:BPC, (q - 1) * 3:q * 3])

        # ---------------- mask ----------------
        nc.gpsimd.iota(jplane_i[:], [[1, W]], channel_multiplier=0)
        nc.vector.tensor_copy(jplane[:], jplane_i[:])
        nc.vector.tensor_copy(Lf[:], Lsb[:])
        for q in range(QN):
            (nc.sync if q % 2 else nc.scalar).dma_start(
                Lbc[q * BPC:(q + 1) * BPC, 0:1], Lf[:])
        for q in range(QN):
            TS(thr[q * BPC:(q + 1) * BPC, 0:1],
               Lbc[q * BPC:(q + 1) * BPC, 0:1],
               3.0, float(q * W), op0=OP.mult, op1=OP.subtract)
        TS(maskp[:], jplane[:], thr[:, 0:1], None, op0=OP.is_lt)

        # ------------- fused frame-fix + P_inc + mask + store -------------
        for c in range(3):
            x = tmps.tile([128, W], F32, tag="t1")
            V.tensor_scalar_mul(x[:], pv[:, 0:W],
                                Fbc[:, _e(c, 0):_e(c, 0) + 1])
            STT(x[:], pv[:, W:2 * W], Fbc[:, _e(c, 1):_e(c, 1) + 1], x[:],
                op0=OP.mult, op1=OP.add)
            STT(x[:], pv[:, 2 * W:3 * W], Fbc[:, _e(c, 2):_e(c, 2) + 1], x[:],
                op0=OP.mult, op1=OP.add)
            STT(Pmall[:, c * W:(c + 1) * W], x[:],
                Pincb[:, c:c + 1], maskp[:], op0=OP.add, op1=OP.mult)
        nc.sync.dma_start(out[:], Pmall[:])


def _prep_alpha(input):
    # pure indexing: alphaN[r]=psi[r-1], alphaCA[r]=omega[r-1] (0 at r=0),
    # alphaC[r]=phi[r]; blocked (q, b, type, m).
    phi, psi, om = input[:, 0], input[:, 1], input[:, 2]
    z1 = np.zeros((input.shape[0], 1), np.float32)
    aN = np.concatenate([z1, psi[:, :-1]], axis=1)
    aCA = np.concatenate([z1, om[:, :-1]], axis=1)
    alpha = np.stack([aN, aCA, phi], axis=1)          # [B, 3, 512]
    return alpha.reshape(-1, 3, QN, NR).transpose(0, 2, 1, 3)


def _shard_alpha(alpha, i):
    sl = slice(i * BPC, (i + 1) * BPC)
    return np.ascontiguousarray(
        alpha[sl].transpose(1, 0, 2, 3).reshape(QN * BPC, 3 * NR))


def _get_nc():
    if "nc" not in _CACHE:
        _CACHE["nc"] = _build_graph()
    return _CACHE["nc"]


def kernel(input, param, angles_length, trace=False):
    input = np.ascontiguousarray(input, dtype=np.float32)
    param = np.ascontiguousarray(param, dtype=np.float32)
    angles_length = np.ascontiguousarray(angles_length, dtype=np.int32)
    nc = _get_nc()
    alpha = _prep_alpha(input)
    in_maps = []
    for i in range(NCORES):
        sl = slice(i * BPC, (i + 1) * BPC)
        in_maps.append({
            "input": _shard_alpha(alpha, i),
            "param": param,
            "angles_length": angles_length[sl],
        })
    res = run_bass_kernel_spmd(nc, in_maps, core_ids=list(range(NCORES)),
                               trace=trace)
    outs = []
    for i in range(NCORES):
        r = res.results[i]["out"]          # [(q,b), (c,j)]
        r = r.reshape(QN, BPC, 3, W)
        r = np.transpose(r, (1, 0, 3, 2)).reshape(BPC, 3 * QN * W)
        outs.append(r)
    full = np.concatenate(outs, axis=0).astype(np.float32)
    if trace:
        kernel._last_exec_ns = res.exec_time_ns
    return full


kernel._last_exec_ns = None


# revision 40
# speedup vs baseline: 1.1291x; 1.0142x over previous
"""Trainium2 Bass kernel for Angles2Backbone (NeRF chain forward).

Full inputs: input [256,3,512] f32, param [6] f32, angles_length [256] i32.
Output: [256, 4608] f32  (coords of 1536 backbone atoms x 3, masked).

Sharding: pure data parallel over batch - 32 proteins per core x 8 cores.

Per-core algorithm (v2, residue-granularity scan):
  - Layout: 128 partitions = (quarter q in 0..3)*32 + protein b. Each row
    owns 128 consecutive residues (=384 atoms) of protein b's chain.
  - Pre-pass: per-residue product Rres = B_N @ B_CA @ B_C computed from
    cos/sin planes with per-type param scalars folded in (leaf structure
    collapses most terms into tensor_scalar/scalar_tensor_tensor ops).
  - Rotation prefix over 128 residues via Hillis-Steele (7 steps) on 9
    entry planes, ping-pong buffered, DVE/Pool split.
  - Cross-quarter fixup: gather quarter-end matrices, 3-step mini-scan,
    apply incoming prefix as per-partition scalars.
  - Atom translations: u_a = R_a * (first column of atom-level prefix)
    expands from residue prefixes with precomputed v-vectors; per-row
    prefix sum via hardware tensor_tensor_scan; additive cross-quarter
    fixup + length mask fused into the final store.
"""

import sys

sys.path.insert(0, "/opt/trn_rl_repo")

import numpy as np
import concourse.bass as bass
import concourse.bacc as bacc
import concourse.mybir as mybir
from concourse import tile
from concourse.bass_utils import run_bass_kernel_spmd

F32 = mybir.dt.float32
I32 = mybir.dt.int32
AF = mybir.ActivationFunctionType
OP = mybir.AluOpType

NCORES = 8
BPC = 32          # proteins per core
L = 512           # residues per protein
QN = 4            # chain quarters per protein (partition groups)
W = 384           # atoms per quarter
NR = 128          # residues per quarter (scan length)
PI = float(np.pi)

_CACHE = {}


def _e(i, k):
    return 3 * i + k


def _build_graph():
    nc = bacc.Bacc("TRN2", target_bir_lowering=False, debug=False,
                   num_devices=NCORES)
    inp = nc.dram_tensor("input", [QN * BPC, 3 * NR], F32,
                     kind="ExternalInput").ap()
    par = nc.dram_tensor("param", [6], F32, kind="ExternalInput").ap()
    alen = nc.dram_tensor("angles_length", [BPC], I32,
                          kind="ExternalInput").ap()
    out = nc.dram_tensor("out", [QN * BPC, 3 * W], F32,
                     kind="ExternalOutput").ap()

    with tile.TileContext(nc) as tc:
        _emit(nc, tc, inp, par, alen, out)
    nc.compile()
    return nc


def _emit(nc, tc, inp, par, alen, out):
    import contextlib
    ctx = contextlib.ExitStack()
    with ctx:
        main = ctx.enter_context(tc.tile_pool(name="main", bufs=1))
        tmps = ctx.enter_context(tc.tile_pool(name="tmps", bufs=18))

        # ---------------- persistent tiles ----------------
        alpha = main.tile([128, W], F32, tag="alpha")
        ca = main.tile([128, W], F32, tag="ca")
        sa = main.tile([128, W], F32, tag="sa")
        C1 = main.tile([128, 9 * NR], F32, tag="C1")
        RA = main.tile([128, 9 * NR], F32, tag="RA")   # residue mats / Pfull
        RB = main.tile([128, 9 * NR], F32, tag="RB")   # ping-pong
        PP = main.tile([128, 4 * NR], F32, tag="PP")   # pp1..pp4
        QQ = main.tile([128, 6 * NR], F32, tag="QQ")   # q1_i, q2_i
        Vm = main.tile([128, 9 * NR], F32, tag="Vm")   # v1,v2,v3 x 3 coords
        zeros = main.tile([128, W], F32, tag="zeros")
        ones = main.tile([128, NR], F32, tag="ones")
        Pall = main.tile([128, 3 * W], F32, tag="Pall")
        Pmall = main.tile([128, 3 * W], F32, tag="Pmall")
        jplane_i = main.tile([128, W], I32, tag="jplane_i")
        jplane = main.tile([128, W], F32, tag="jplane")
        maskp = main.tile([128, W], F32, tag="maskp")
        thr = main.tile([128, 1], F32, tag="thr")
        Lbc = main.tile([128, 1], F32, tag="Lbc")
        Lsb = main.tile([BPC, 1], I32, tag="Lsb")
        Lf = main.tile([BPC, 1], F32, tag="Lf")
        Psb = main.tile([1, 6], F32, tag="Psb")
        kv = main.tile([1, 3], F32, tag="kv")
        Rv = main.tile([1, 3], F32, tag="Rv")
        NSC = 24
        vecs = main.tile([1, NSC], F32, tag="vecs")
        Vb = main.tile([128, NSC], F32, tag="Vb")
        Estack = main.tile([BPC, 36], F32, tag="Estack")
        Fstack = main.tile([BPC, 27], F32, tag="Fstack")
        Fbc = main.tile([128, 9], F32, tag="Fbc")
        pestage = main.tile([BPC, 9], F32, tag="pestage")
        cumst = main.tile([BPC, 9], F32, tag="cumst")
        Pincb = main.tile([128, 3], F32, tag="Pincb")
        zb1 = main.tile([1, 1], F32, tag="zb1")
        zb128 = main.tile([128, 1], F32, tag="zb128")

        _cnt = [0]

        def ENG():
            # TT ops only: alternate DVE (2/3) and Pool (1/3)
            _cnt[0] += 1
            return nc.gpsimd if (_cnt[0] % 3 == 0) else nc.vector

        # ---------------- input DMA: assemble alpha ----------------
        # inp viewed as [c][q][b][m] so one DMA covers all 4 quarters
        # (dst partition order is (q, b), matching the AP walk order).
        av = alpha[:]
        nc.scalar.dma_start(Psb[:], par[:])
        nc.scalar.dma_start(Lsb[:], alen[:])
        nc.sync.dma_start(av[:, :], inp[:])

        # ---------------- param scalars ----------------
        for t, idx in enumerate((5, 1, 3)):   # kappa: CA_C_N, C_N_CA, N_CA_C
            nc.vector.tensor_copy(kv[0:1, t:t + 1], Psb[0:1, idx:idx + 1])
        for t, idx in enumerate((4, 0, 2)):   # R: R_C_N, R_N_CA, R_CA_C
            nc.vector.tensor_copy(Rv[0:1, t:t + 1], Psb[0:1, idx:idx + 1])
        nc.vector.memset(zb1[:], 0.0)
        nc.vector.memset(zb128[:], 0.0)
        # per-type ck/sk: sk=sin(kappa) (kappa in (0,pi)); ck=1-2sin^2(k/2)
        sk3 = main.tile([1, 3], F32, tag="sk3")
        ck3 = main.tile([1, 3], F32, tag="ck3")
        kvr = main.tile([1, 3], F32, tag="kvr")
        nc.scalar.activation(sk3[:], kv[0:1, 0:3], AF.Sin, bias=zb1[:])
        nc.scalar.activation(kvr[:], kv[0:1, 0:3], AF.Sin, bias=zb1[:],
                             scale=0.5)
        nc.scalar.square(kvr[:], kvr[:])
        nc.vector.tensor_scalar(ck3[:], kvr[:], -2.0, 1.0,
                                op0=OP.mult, op1=OP.add)

        # scalar slot layout in vecs[1, NSC]:
        # 0:ckN 1:skN 2:ckA 3:skA 4:ckC 5:skC
        # 6:ckNckA 7:skNskA 8:ckNskA 9:skNckA
        # 10:nskNckA 11:nckNskA 12:nckN 13:nckA 14:nckC 15:nskA
        # 16:RNckN 17:RNskN 18:RCA 19:RC
        def vc(i):
            return vecs[0:1, i:i + 1]

        for t in range(3):
            nc.vector.tensor_copy(vc(2 * t), ck3[0:1, t:t + 1])
            nc.vector.tensor_copy(vc(2 * t + 1), sk3[0:1, t:t + 1])
        nc.vector.tensor_mul(vc(6), vc(0), vc(2))     # ckN*ckA
        nc.vector.tensor_mul(vc(7), vc(1), vc(3))     # skN*skA
        nc.vector.tensor_mul(vc(8), vc(0), vc(3))     # ckN*skA
        nc.vector.tensor_mul(vc(9), vc(1), vc(2))     # skN*ckA
        nc.vector.tensor_scalar_mul(vc(10), vc(9), -1.0)
        nc.vector.tensor_scalar_mul(vc(11), vc(8), -1.0)
        nc.vector.tensor_scalar_mul(vc(12), vc(0), -1.0)
        nc.vector.tensor_scalar_mul(vc(13), vc(2), -1.0)
        nc.vector.tensor_scalar_mul(vc(14), vc(4), -1.0)
        nc.vector.tensor_scalar_mul(vc(15), vc(3), -1.0)
        nc.vector.tensor_mul(vc(16), Rv[0:1, 0:1], vc(0))   # RN*ckN
        nc.vector.tensor_mul(vc(17), Rv[0:1, 0:1], vc(1))   # RN*skN
        nc.vector.tensor_copy(vc(18), Rv[0:1, 1:2])         # R_CA
        nc.vector.tensor_copy(vc(19), Rv[0:1, 2:3])         # R_C
        nc.gpsimd.partition_broadcast(Vb[:], vecs[:])

        S = {}
        for i, nm in enumerate(("ckN", "skN", "ckA", "skA", "ckC", "skC",
                                "ckNckA", "skNskA", "ckNskA", "skNckA",
                                "nskNckA", "nckNskA", "nckN", "nckA",
                                "nckC", "nskA", "RNckN", "RNskN",
                                "RCA", "RC")):
            S[nm] = Vb[:, i:i + 1]

        # trig: |alpha| < 4pi: s4=sin(a/4), c4=1-2sin^2(a/8);
        # s2=2*s4*c4, c2=1-2*s4^2; s1=2*s2*c2, c1=1-2*s2^2.
        # One chain per 128-col type block, pipelined across ACT/DVE.
        for t in range(3):
            bs = slice(t * NR, (t + 1) * NR)
            avb, cab, sab = av[:, bs], ca[:, bs], sa[:, bs]
            ts8 = tmps.tile([128, NR], F32, tag="t1")
            ts4 = tmps.tile([128, NR], F32, tag="t2")
            tq = tmps.tile([128, NR], F32, tag="t1")
            nc.scalar.activation(ts8[:], avb, AF.Sin, bias=zb128[:],
                                 scale=0.125)
            nc.scalar.activation(ts4[:], avb, AF.Sin, bias=zb128[:],
                                 scale=0.25)
            nc.scalar.square(ts8[:], ts8[:])
            nc.vector.tensor_scalar(cab, ts8[:], -2.0, 1.0,
                                    op0=OP.mult, op1=OP.add)          # c4
            nc.vector.scalar_tensor_tensor(ts8[:], ts4[:], 2.0, cab,
                                           op0=OP.mult, op1=OP.mult)  # s2
            nc.scalar.square(tq[:], ts4[:])
            nc.vector.tensor_scalar(ts4[:], tq[:], -2.0, 1.0,
                                    op0=OP.mult, op1=OP.add)          # c2
            nc.vector.scalar_tensor_tensor(sab, ts8[:], 2.0, ts4[:],
                                           op0=OP.mult, op1=OP.mult)  # s1
            nc.scalar.square(tq[:], ts8[:])
            nc.vector.tensor_scalar(cab, tq[:], -2.0, 1.0,
                                    op0=OP.mult, op1=OP.add)          # c1
        nc.gpsimd.memset(zeros[:], 0.0)
        nc.gpsimd.memset(zeros[:], 0.0)
        nc.gpsimd.memset(ones[:], 1.0)

        # per-type strided cos/sin views [128, 128]
        cN, sN = ca[:, 0:128], sa[:, 0:128]
        cA, sA = ca[:, 128:256], sa[:, 128:256]
        cC, sC = ca[:, 256:384], sa[:, 256:384]

        def blk(t, e, lo=0, hi=NR):
            return t[:, e * NR + lo:e * NR + hi]

        V = nc.vector
        STT = nc.vector.scalar_tensor_tensor
        TS = nc.vector.tensor_scalar

        # ---------------- pre-pass: C1 = B_N @ B_CA ----------------
        pp1 = PP[:, 0 * NR:1 * NR]
        pp2 = PP[:, 1 * NR:2 * NR]
        pp3 = PP[:, 2 * NR:3 * NR]
        pp4 = PP[:, 3 * NR:4 * NR]
        nc.gpsimd.tensor_mul(pp1, cN, cA)
        nc.gpsimd.tensor_mul(pp2, sN, sA)
        nc.gpsimd.tensor_mul(pp3, cN, sA)
        nc.gpsimd.tensor_mul(pp4, sN, cA)
        c1 = C1[:]
        TS(blk(c1, 0), cA, S["skNskA"], S["ckNckA"],
           op0=OP.mult, op1=OP.add)                       # C1_00
        TS(blk(c1, 1), cA, S["nskNckA"], S["ckNskA"],
           op0=OP.mult, op1=OP.add)                       # C1_01
        V.tensor_scalar_mul(blk(c1, 2), sA, S["skN"])     # C1_02
        x = blk(c1, 3)                                    # C1_10
        nc.scalar.mul(x, cN, S["skNckA"])
        STT(x, pp1, S["nckNskA"], x, op0=OP.mult, op1=OP.add)
        STT(x, pp2, S["skA"], x, op0=OP.mult, op1=OP.add)
        x = blk(c1, 4)                                    # C1_11
        nc.scalar.mul(x, cN, S["skNskA"])
        STT(x, pp1, S["ckNckA"], x, op0=OP.mult, op1=OP.add)
        STT(x, pp2, S["nckA"], x, op0=OP.mult, op1=OP.add)
        STT(blk(c1, 5), pp3, S["nckN"], pp4,
            op0=OP.mult, op1=OP.subtract)                 # C1_12
        x = blk(c1, 6)                                    # C1_20
        nc.scalar.mul(x, sN, S["skNckA"])
        STT(x, pp4, S["nckNskA"], x, op0=OP.mult, op1=OP.add)
        STT(x, pp3, S["nskA"], x, op0=OP.mult, op1=OP.add)
        x = blk(c1, 7)                                    # C1_21
        nc.scalar.mul(x, sN, S["skNskA"])
        STT(x, pp4, S["ckNckA"], x, op0=OP.mult, op1=OP.add)
        STT(x, pp3, S["ckA"], x, op0=OP.mult, op1=OP.add)
        STT(blk(c1, 8), pp2, S["nckN"], pp1,
            op0=OP.mult, op1=OP.add)                      # C1_22

        # residue-0 of q=0: B_N := Identity => C1 := B_CA(0)
        # (alpha_CA(0)=0 so cA=1, sA=0 there): [[ckA,skA,0],[skA,-ckA,0],
        # [0,0,-1]]
        r0s = slice(0, BPC)
        o1 = ones[r0s, 0:1]
        V.tensor_scalar_mul(c1[r0s, 0 * NR:0 * NR + 1], o1, S["ckA"][r0s])
        V.tensor_scalar_mul(c1[r0s, 1 * NR:1 * NR + 1], o1, S["skA"][r0s])
        V.memset(c1[r0s, 2 * NR:2 * NR + 1], 0.0)
        V.tensor_scalar_mul(c1[r0s, 3 * NR:3 * NR + 1], o1, S["skA"][r0s])
        V.tensor_scalar_mul(c1[r0s, 4 * NR:4 * NR + 1], o1, S["nckA"][r0s])
        V.memset(c1[r0s, 5 * NR:5 * NR + 1], 0.0)
        V.memset(c1[r0s, 6 * NR:6 * NR + 1], 0.0)
        V.memset(c1[r0s, 7 * NR:7 * NR + 1], 0.0)
        V.memset(c1[r0s, 8 * NR:8 * NR + 1], -1.0)

        # ---------------- pre-pass: Rres = C1 @ B_C -> RA ----------------
        ra = RA[:]
        for i in range(3):
            nc.gpsimd.tensor_mul(blk(QQ[:], i), blk(c1, _e(i, 1)), cC)
            nc.gpsimd.tensor_mul(blk(QQ[:], 3 + i), blk(c1, _e(i, 2)), sC)
        for i in range(3):
            q1i = blk(QQ[:], i)
            q2i = blk(QQ[:], 3 + i)
            x = blk(ra, _e(i, 0))
            nc.scalar.mul(x, blk(c1, _e(i, 0)), S["ckC"])
            STT(x, q1i, S["skC"], x, op0=OP.mult, op1=OP.add)
            STT(x, q2i, S["skC"], x, op0=OP.mult, op1=OP.add)
            x = blk(ra, _e(i, 1))
            nc.scalar.mul(x, blk(c1, _e(i, 0)), S["skC"])
            STT(x, q1i, S["nckC"], x, op0=OP.mult, op1=OP.add)
            STT(x, q2i, S["nckC"], x, op0=OP.mult, op1=OP.add)
            t1 = tmps.tile([128, NR], F32, tag="pt1")
            t2 = tmps.tile([128, NR], F32, tag="pt2")
            nc.gpsimd.tensor_mul(t1[:], blk(c1, _e(i, 1)), sC)
            nc.gpsimd.tensor_mul(t2[:], blk(c1, _e(i, 2)), cC)
            nc.gpsimd.tensor_sub(blk(ra, _e(i, 2)), t1[:], t2[:])

        # ---------------- v-vectors for atom expansion ----------------
        # v1 = t_N = RN*(ckN, skN*cN, skN*sN); v2 = RCA*C1[:,0];
        # v3 = RC*Rres[:,0]
        vm = Vm[:]
        nc.scalar.mul(blk(vm, 0), ones[:], S["RNckN"])
        nc.scalar.mul(blk(vm, 1), cN, S["RNskN"])
        nc.scalar.mul(blk(vm, 2), sN, S["RNskN"])
        for i in range(3):
            nc.scalar.mul(blk(vm, 3 + i), blk(c1, _e(i, 0)), S["RCA"])
            nc.scalar.mul(blk(vm, 6 + i), blk(ra, _e(i, 0)), S["RC"])

        # ---------------- Hillis-Steele residue scan ----------------
        # Fused step: all 9 output entries in one 3-dim AP op per k-term:
        #   out[i,j] += L[i,k] (bcast over j) * R[k,j] (bcast over i)
        # 5 logical ops per step, each split col-wise DVE/Pool.
        def ap3(base_ap, off, dims):
            return bass.AP(base_ap.tensor, base_ap.offset + off,
                           [list(base_ap.ap[0])] + [list(d) for d in dims])

        T9a = main.tile([128, 9 * NR], F32, tag="T9a")
        T9b = main.tile([128, 9 * NR], F32, tag="T9b")

        def fused_step(srcb, dstb, s, nr):
            n = nr - s
            cut = (n * 5) // 8          # DVE share of columns
            sv = srcb.rearrange("p (e j) -> p e j", e=9)
            dv = dstb.rearrange("p (e j) -> p e j", e=9)
            nc.scalar.copy(dv[:, :, 0:s], sv[:, :, 0:s])

            def L(k, c0, c1):
                return ap3(srcb, k * nr + c0,
                           [[3 * nr, 3], [0, 3], [1, c1 - c0]])

            def R(k, c0, c1):
                return ap3(srcb, 3 * k * nr + s + c0,
                           [[0, 3], [nr, 3], [1, c1 - c0]])

            def T(t, c0, c1):
                return ap3(t[:], c0, [[3 * nr, 3], [nr, 3], [1, c1 - c0]])

            def O(c0, c1):
                return ap3(dstb, s + c0, [[3 * nr, 3], [nr, 3], [1, c1 - c0]])

            for E, c0, c1 in ((nc.vector, 0, cut), (nc.gpsimd, cut, n)):
                if c1 <= c0:
                    continue
                E.tensor_mul(T(T9a, c0, c1), L(0, c0, c1), R(0, c0, c1))
                E.tensor_mul(T(T9b, c0, c1), L(1, c0, c1), R(1, c0, c1))
                E.tensor_add(T(T9a, c0, c1), T(T9a, c0, c1), T(T9b, c0, c1))
                E.tensor_mul(T(T9b, c0, c1), L(2, c0, c1), R(2, c0, c1))
                E.tensor_add(O(c0, c1), T(T9a, c0, c1), T(T9b, c0, c1))

        # pair adjacent residues: P2[r'] = Rres[2r'] @ Rres[2r'+1]
        NR2 = NR // 2
        P2A = main.tile([128, 9 * NR2], F32, tag="P2A")
        P2B = main.tile([128, 9 * NR2], F32, tag="P2B")
        pcut = (NR2 * 5) // 8
        ra_ap = RA[:]

        def PL(k, c0, c1):
            return ap3(ra_ap, k * NR + 2 * c0,
                       [[3 * NR, 3], [0, 3], [2, c1 - c0]])

        def PR(k, c0, c1):
            return ap3(ra_ap, 3 * k * NR + 1 + 2 * c0,
                       [[0, 3], [NR, 3], [2, c1 - c0]])

        def PT(t, c0, c1):
            return ap3(t[:], c0, [[3 * NR2, 3], [NR2, 3], [1, c1 - c0]])

        def PO(c0, c1):
            return ap3(P2A[:], c0, [[3 * NR2, 3], [NR2, 3], [1, c1 - c0]])

        for E, c0, c1 in ((nc.vector, 0, pcut), (nc.gpsimd, pcut, NR2)):
            E.tensor_mul(PT(T9a, c0, c1), PL(0, c0, c1), PR(0, c0, c1))
            E.tensor_mul(PT(T9b, c0, c1), PL(1, c0, c1), PR(1, c0, c1))
            E.tensor_add(PT(T9a, c0, c1), PT(T9a, c0, c1), PT(T9b, c0, c1))
            E.tensor_mul(PT(T9b, c0, c1), PL(2, c0, c1), PR(2, c0, c1))
            E.tensor_add(PO(c0, c1), PT(T9a, c0, c1), PT(T9b, c0, c1))

        Wodd = main.tile([128, 9 * NR2], F32, tag="Wodd")
        T9c = main.tile([128, 9 * NR2], F32, tag="T9c")
        T9d = main.tile([128, 9 * NR2], F32, tag="T9d")
        wo = Wodd[:]
        wcut = (NR2 * 5) // 8

        def WL(k, c0, c1):
            return ap3(ra_ap, k * NR + 2 * c0,
                       [[3 * NR, 3], [0, 3], [2, c1 - c0]])

        def WR(k, c0, c1):
            return ap3(vm, k * NR + 1 + 2 * c0,
                       [[0, 3], [3 * NR, 3], [2, c1 - c0]])

        def WT(t, c0, c1):
            return ap3(t[:], c0, [[3 * NR2, 3], [NR2, 3], [1, c1 - c0]])

        def WO(c0, c1):
            return ap3(wo, c0, [[NR2, 3], [3 * NR2, 3], [1, c1 - c0]])

        for E, c0, c1 in ((nc.vector, 0, wcut), (nc.gpsimd, wcut, NR2)):
            E.tensor_mul(WT(T9a, c0, c1), WL(0, c0, c1), WR(0, c0, c1))
            E.tensor_mul(WT(T9b, c0, c1), WL(1, c0, c1), WR(1, c0, c1))
            E.tensor_add(WT(T9a, c0, c1), WT(T9a, c0, c1), WT(T9b, c0, c1))
            E.tensor_mul(WT(T9b, c0, c1), WL(2, c0, c1), WR(2, c0, c1))
            E.tensor_add(WO(c0, c1), WT(T9a, c0, c1), WT(T9b, c0, c1))


        bufs = [P2A, P2B]
        nsteps = 6
        for step in range(nsteps):
            fused_step(bufs[step % 2][:], bufs[(step + 1) % 2][:],
                       1 << step, NR2)
        Rscan = bufs[nsteps % 2][:]    # RB: local residue prefixes

        # ---------------- cross-quarter rotation fixup ----------------
        for q in range(QN):
            (nc.sync if q % 2 else nc.scalar).dma_start(
                Estack[0:BPC, q * 9:(q + 1) * 9],
                Rscan[q * BPC:(q + 1) * BPC, NR2 - 1:9 * NR2:NR2])
        nc.vector.tensor_copy(Fstack[0:BPC, 0:9], Estack[0:BPC, 0:9])
        mt0 = main.tile([BPC, 9], F32, tag="mt0")
        mt1 = main.tile([BPC, 9], F32, tag="mt1")
        fs = Fstack[:]
        es = Estack[:]

        def ap2(base_ap, off, dims):
            return bass.AP(base_ap.tensor, base_ap.offset + off,
                           [list(base_ap.ap[0])] + [list(d) for d in dims])

        for q in (1, 2):
            FL = lambda k: ap2(fs, (q - 1) * 9 + k, [[3, 3], [0, 3]])
            ER = lambda k: ap2(es, q * 9 + 3 * k, [[0, 3], [1, 3]])
            MT = lambda t: ap2(t[:], 0, [[3, 3], [1, 3]])
            FO = ap2(fs, q * 9, [[3, 3], [1, 3]])
            V.tensor_mul(MT(mt0), FL(0), ER(0))
            V.tensor_mul(MT(mt1), FL(1), ER(1))
            V.tensor_add(MT(mt0), MT(mt0), MT(mt1))
            V.tensor_mul(MT(mt1), FL(2), ER(2))
            V.tensor_add(FO, MT(mt0), MT(mt1))
        nc.vector.memset(Fbc[0:BPC, 0:9], 0.0)
        for e in (0, 4, 8):
            nc.vector.memset(Fbc[0:BPC, e:e + 1], 1.0)
        for q in (1, 2, 3):
            (nc.sync if q % 2 else nc.scalar).dma_start(
                Fbc[q * BPC:(q + 1) * BPC, 0:9],
                              Fstack[0:BPC, (q - 1) * 9:q * 9])
        # ---------------- atom translations (local frame) ----------------
        # superblock = 2 residues = 6 atoms. w_m (m=0..5): prefix-within-
        # superblock applied to t-vectors; m<3 are the per-residue v's at
        # even residues, m>=3 need Rres_even @ v_odd (fused below).
        Uloc = main.tile([128, 3 * W], F32, tag="Uloc")
        ul = Uloc[:]
        rs = Rscan
        # superblock 0: local prefix = identity -> u = w_m
        V.tensor_copy(ap3(ul, 0, [[W, 3], [1, 3]]),
                      ap3(vm, 0, [[NR, 3], [3 * NR, 3]]))
        V.tensor_copy(ap3(ul, 3, [[W, 3], [1, 3]]),
                      ap3(wo, 0, [[NR2, 3], [3 * NR2, 3]]))
        nu = NR2 - 1
        ucut = (nu * 5) // 8

        def UL(k, c0, c1):
            return ap3(rs, k * NR2 + c0, [[3 * NR2, 3], [0, 3], [1, c1 - c0]])

        def UREV(k, c0, c1):   # even residues 2r', r'=1..63
            return ap3(vm, k * NR + 2 + 2 * c0,
                       [[0, 3], [3 * NR, 3], [2, c1 - c0]])

        def UROD(k, c0, c1):   # w_{3..5} at superblocks 1..63
            return ap3(wo, k * NR2 + 1 + c0,
                       [[0, 3], [3 * NR2, 3], [1, c1 - c0]])

        def UT(t, c0, c1):
            return ap3(t[:], c0, [[3 * NR2, 3], [NR2, 3], [1, c1 - c0]])

        def UO(off, c0, c1):
            return ap3(ul, off + 6 * c0, [[W, 3], [1, 3], [6, c1 - c0]])

        for E, c0, c1 in ((nc.vector, 0, ucut), (nc.gpsimd, ucut, nu)):
            E.tensor_mul(UT(T9c, c0, c1), UL(0, c0, c1), UREV(0, c0, c1))
            E.tensor_mul(UT(T9d, c0, c1), UL(1, c0, c1), UREV(1, c0, c1))
            E.tensor_add(UT(T9c, c0, c1), UT(T9c, c0, c1), UT(T9d, c0, c1))
            E.tensor_mul(UT(T9d, c0, c1), UL(2, c0, c1), UREV(2, c0, c1))
            E.tensor_add(UO(6, c0, c1), UT(T9c, c0, c1), UT(T9d, c0, c1))
        for E, c0, c1 in ((nc.vector, 0, ucut), (nc.gpsimd, ucut, nu)):
            E.tensor_mul(UT(T9a, c0, c1), UL(0, c0, c1), UROD(0, c0, c1))
            E.tensor_mul(UT(T9b, c0, c1), UL(1, c0, c1), UROD(1, c0, c1))
            E.tensor_add(UT(T9a, c0, c1), UT(T9a, c0, c1), UT(T9b, c0, c1))
            E.tensor_mul(UT(T9b, c0, c1), UL(2, c0, c1), UROD(2, c0, c1))
            E.tensor_add(UO(9, c0, c1), UT(T9a, c0, c1), UT(T9b, c0, c1))
        # prefix-sum the LOCAL u per coordinate (frame fix applied at the
        # end by linearity: sum_j F@u = F@sum_j u)
        for c in range(3):
            uc = ul[:, c * W:(c + 1) * W]
            V.memset(uc[0:BPC, 0:1], 0.0)   # atom 0 of the whole chain
            nc.vector.tensor_tensor_scan(
                Pall[:, c * W:(c + 1) * W], uc, zeros[:], 0.0,
                op0=OP.add, op1=OP.add)

        # ---------------- cross-quarter translation fixup ----------------
        pv = Pall[:]
        for q in range(3):
            (nc.sync if q % 2 else nc.scalar).dma_start(
                pestage[0:BPC, q * 3:(q + 1) * 3],
                              pv[q * BPC:(q + 1) * BPC, W - 1:3 * W:W])
        # global pe_q = F_q @ pe_local_q (F_0 = I); Fstack block q-1 = F_q
        peg = main.tile([BPC, 9], F32, tag="peg")
        ps = pestage[:]
        nc.vector.tensor_copy(peg[0:BPC, 0:3], pestage[0:BPC, 0:3])
        for q in (1, 2):
            FL = lambda k: ap2(fs, (q - 1) * 9 + k, [[3, 3]])
            PR = lambda k: ap2(ps, q * 3 + k, [[0, 3]])
            M3 = lambda t: ap2(t[:], 0, [[1, 3]])
            PO = ap2(peg[:], q * 3, [[1, 3]])
            V.tensor_mul(M3(mt0), FL(0), PR(0))
            V.tensor_mul(M3(mt1), FL(1), PR(1))
            V.tensor_add(M3(mt0), M3(mt0), M3(mt1))
            V.tensor_mul(M3(mt1), FL(2), PR(2))
            V.tensor_add(PO, M3(mt0), M3(mt1))
        nc.vector.tensor_copy(cumst[0:BPC, 0:3], peg[0:BPC, 0:3])
        nc.vector.tensor_add(cumst[0:BPC, 3:6], cumst[0:BPC, 0:3],
                             peg[0:BPC, 3:6])
        nc.vector.tensor_add(cumst[0:BPC, 6:9], cumst[0:BPC, 3:6],
                             peg[0:BPC, 6:9])
        nc.vector.memset(Pincb[0:BPC, 0:3], 0.0)
        for q in (1, 2, 3):
            (nc.sync if q % 2 else nc.scalar).dma_start(
                Pincb[q * BPC:(q + 1) * BPC, 0:3],
                              cumst[0:BPC, (q - 1) * 3:q * 3])

        # ---------------- mask ----------------
        nc.gpsimd.iota(jplane_i[:], [[1, W]], channel_multiplier=0)
        nc.vector.tensor_copy(jplane[:], jplane_i[:])
        nc.vector.tensor_copy(Lf[:], Lsb[:])
        for q in range(QN):
            (nc.sync if q % 2 else nc.scalar).dma_start(
                Lbc[q * BPC:(q + 1) * BPC, 0:1], Lf[:])
        for q in range(QN):
            TS(thr[q * BPC:(q + 1) * BPC, 0:1],
               Lbc[q * BPC:(q + 1) * BPC, 0:1],
               3.0, float(q * W), op0=OP.mult, op1=OP.subtract)
        TS(maskp[:], jplane[:], thr[:, 0:1], None, op0=OP.is_lt)

        # ------------- fused frame-fix + P_inc + mask + store -------------
        for c in range(3):
            x = tmps.tile([128, W], F32, tag="t1")
            V.tensor_scalar_mul(x[:], pv[:, 0:W],
                                Fbc[:, _e(c, 0):_e(c, 0) + 1])
            STT(x[:], pv[:, W:2 * W], Fbc[:, _e(c, 1):_e(c, 1) + 1], x[:],
                op0=OP.mult, op1=OP.add)
            STT(x[:], pv[:, 2 * W:3 * W], Fbc[:, _e(c, 2):_e(c, 2) + 1], x[:],
                op0=OP.mult, op1=OP.add)
            STT(Pmall[:, c * W:(c + 1) * W], x[:],
                Pincb[:, c:c + 1], maskp[:], op0=OP.add, op1=OP.mult)
        nc.sync.dma_start(out[:], Pmall[:])


def _prep_alpha(input):
    # pure indexing: alphaN[r]=psi[r-1], alphaCA[r]=omega[r-1] (0 at r=0),
    # alphaC[r]=phi[r]; blocked (q, b, type, m).
    phi, psi, om = input[:, 0], input[:, 1], input[:, 2]
    z1 = np.zeros((input.shape[0], 1), np.float32)
    aN = np.concatenate([z1, psi[:, :-1]], axis=1)
    aCA = np.concatenate([z1, om[:, :-1]], axis=1)
    alpha = np.stack([aN, aCA, phi], axis=1)          # [B, 3, 512]
    return alpha.reshape(-1, 3, QN, NR).transpose(0, 2, 1, 3)


def _shard_alpha(alpha, i):
    sl = slice(i * BPC, (i + 1) * BPC)
    return np.ascontiguousarray(
        alpha[sl].transpose(1, 0, 2, 3).reshape(QN * BPC, 3 * NR))


def _get_nc():
    if "nc" not in _CACHE:
        _CACHE["nc"] = _build_graph()
    return _CACHE["nc"]


def kernel(input, param, angles_length, trace=False):
    input = np.ascontiguousarray(input, dtype=np.float32)
    param = np.ascontiguousarray(param, dtype=np.float32)
    angles_length = np.ascontiguousarray(angles_length, dtype=np.int32)
    nc = _get_nc()
    alpha = _prep_alpha(input)
    in_maps = []
    for i in range(NCORES):
        sl = slice(i * BPC, (i + 1) * BPC)
        in_maps.append({
            "input": _shard_alpha(alpha, i),
            "param": param,
            "angles_length": angles_length[sl],
        })
    res = run_bass_kernel_spmd(nc, in_maps, core_ids=list(range(NCORES)),
                               trace=trace)
    outs = []
    for i in range(NCORES):
        r = res.results[i]["out"]          # [(q,b), (c,j)]
        r = r.reshape(QN, BPC, 3, W)
        r = np.transpose(r, (1, 0, 3, 2)).reshape(BPC, 3 * QN * W)
        outs.append(r)
    full = np.concatenate(outs, axis=0).astype(np.float32)
    if trace:
        kernel._last_exec_ns = res.exec_time_ns
    return full


kernel._last_exec_ns = None


# revision 45
# speedup vs baseline: 1.1770x; 1.0424x over previous
"""Trainium2 Bass kernel for Angles2Backbone (NeRF chain forward).

Full inputs: input [256,3,512] f32, param [6] f32, angles_length [256] i32.
Output: [256, 4608] f32  (coords of 1536 backbone atoms x 3, masked).

Sharding: pure data parallel over batch - 32 proteins per core x 8 cores.

Per-core algorithm (v2, residue-granularity scan):
  - Layout: 128 partitions = (quarter q in 0..3)*32 + protein b. Each row
    owns 128 consecutive residues (=384 atoms) of protein b's chain.
  - Pre-pass: per-residue product Rres = B_N @ B_CA @ B_C computed from
    cos/sin planes with per-type param scalars folded in (leaf structure
    collapses most terms into tensor_scalar/scalar_tensor_tensor ops).
  - Rotation prefix over 128 residues via Hillis-Steele (7 steps) on 9
    entry planes, ping-pong buffered, DVE/Pool split.
  - Cross-quarter fixup: gather quarter-end matrices, 3-step mini-scan,
    apply incoming prefix as per-partition scalars.
  - Atom translations: u_a = R_a * (first column of atom-level prefix)
    expands from residue prefixes with precomputed v-vectors; per-row
    prefix sum via hardware tensor_tensor_scan; additive cross-quarter
    fixup + length mask fused into the final store.
"""

import sys

sys.path.insert(0, "/opt/trn_rl_repo")

import numpy as np
import concourse.bass as bass
import concourse.bacc as bacc
import concourse.mybir as mybir
from concourse import tile
from concourse.bass_utils import run_bass_kernel_spmd

F32 = mybir.dt.float32
I32 = mybir.dt.int32
AF = mybir.ActivationFunctionType
OP = mybir.AluOpType

NCORES = 8
BPC = 32          # proteins per core
L = 512           # residues per protein
QN = 4            # chain quarters per protein (partition groups)
W = 384           # atoms per quarter
NR = 128          # residues per quarter (scan length)
PI = float(np.pi)

_CACHE = {}


def _e(i, k):
    return 3 * i + k


def _build_graph():
    nc = bacc.Bacc("TRN2", target_bir_lowering=False, debug=False,
                   num_devices=NCORES)
    inp = nc.dram_tensor("input", [QN * BPC, 3 * NR], F32,
                     kind="ExternalInput").ap()
    par = nc.dram_tensor("param", [6], F32, kind="ExternalInput").ap()
    alen = nc.dram_tensor("angles_length", [BPC], I32,
                          kind="ExternalInput").ap()
    out = nc.dram_tensor("out", [QN * BPC, 3 * W], F32,
                     kind="ExternalOutput").ap()

    with tile.TileContext(nc) as tc:
        _emit(nc, tc, inp, par, alen, out)
    nc.compile()
    return nc


def _emit(nc, tc, inp, par, alen, out):
    import contextlib
    ctx = contextlib.ExitStack()
    with ctx:
        main = ctx.enter_context(tc.tile_pool(name="main", bufs=1))
        tmps = ctx.enter_context(tc.tile_pool(name="tmps", bufs=18))
        psum = ctx.enter_context(tc.tile_pool(name="psum", bufs=1,
                                              space="PSUM"))

        # ---------------- persistent tiles ----------------
        alpha = main.tile([128, W], F32, tag="alpha")
        ca = main.tile([128, W], F32, tag="ca")
        sa = main.tile([128, W], F32, tag="sa")
        C1 = main.tile([128, 9 * NR], F32, tag="C1")
        RA = main.tile([128, 9 * NR], F32, tag="RA")   # residue mats / Pfull
        RB = main.tile([128, 9 * NR], F32, tag="RB")   # ping-pong
        PP = main.tile([128, 4 * NR], F32, tag="PP")   # pp1..pp4
        QQ = main.tile([128, 6 * NR], F32, tag="QQ")   # q1_i, q2_i
        Vm = main.tile([128, 9 * NR], F32, tag="Vm")   # v1,v2,v3 x 3 coords
        zeros = main.tile([128, W], F32, tag="zeros")
        ones = main.tile([128, NR], F32, tag="ones")
        Pall = main.tile([128, 3 * W], F32, tag="Pall")
        Pmall = main.tile([128, 3 * W], F32, tag="Pmall")
        jplane_i = main.tile([128, W], I32, tag="jplane_i")
        jplane = main.tile([128, W], F32, tag="jplane")
        maskp = main.tile([128, W], F32, tag="maskp")
        thr = main.tile([128, 1], F32, tag="thr")
        Lbc = main.tile([128, 1], F32, tag="Lbc")
        Lsb = main.tile([BPC, 1], I32, tag="Lsb")
        Lf = main.tile([BPC, 1], F32, tag="Lf")
        Psb = main.tile([1, 6], F32, tag="Psb")
        kv = main.tile([1, 3], F32, tag="kv")
        Rv = main.tile([1, 3], F32, tag="Rv")
        NSC = 24
        vecs = main.tile([1, NSC], F32, tag="vecs")
        Vb = main.tile([128, NSC], F32, tag="Vb")
        Estack = main.tile([BPC, 36], F32, tag="Estack")
        Fstack = main.tile([BPC, 27], F32, tag="Fstack")
        Fbc = main.tile([128, 9], F32, tag="Fbc")
        pestage = main.tile([BPC, 9], F32, tag="pestage")
        cumst = main.tile([BPC, 9], F32, tag="cumst")
        Pincb = main.tile([128, 3], F32, tag="Pincb")
        zb1 = main.tile([1, 1], F32, tag="zb1")
        zb128 = main.tile([128, 1], F32, tag="zb128")

        _cnt = [0]

        def ENG():
            # TT ops only: alternate DVE (2/3) and Pool (1/3)
            _cnt[0] += 1
            return nc.gpsimd if (_cnt[0] % 3 == 0) else nc.vector

        # ---------------- input DMA: assemble alpha ----------------
        # inp viewed as [c][q][b][m] so one DMA covers all 4 quarters
        # (dst partition order is (q, b), matching the AP walk order).
        av = alpha[:]
        nc.scalar.dma_start(Psb[:], par[:])
        nc.scalar.dma_start(Lsb[:], alen[:])
        nc.sync.dma_start(av[:, :], inp[:])

        # ---------------- param scalars ----------------
        for t, idx in enumerate((5, 1, 3)):   # kappa: CA_C_N, C_N_CA, N_CA_C
            nc.vector.tensor_copy(kv[0:1, t:t + 1], Psb[0:1, idx:idx + 1])
        for t, idx in enumerate((4, 0, 2)):   # R: R_C_N, R_N_CA, R_CA_C
            nc.vector.tensor_copy(Rv[0:1, t:t + 1], Psb[0:1, idx:idx + 1])
        nc.vector.memset(zb1[:], 0.0)
        nc.vector.memset(zb128[:], 0.0)
        # per-type ck/sk: sk=sin(kappa) (kappa in (0,pi)); ck=1-2sin^2(k/2)
        sk3 = main.tile([1, 3], F32, tag="sk3")
        ck3 = main.tile([1, 3], F32, tag="ck3")
        kvr = main.tile([1, 3], F32, tag="kvr")
        nc.scalar.activation(sk3[:], kv[0:1, 0:3], AF.Sin, bias=zb1[:])
        nc.scalar.activation(kvr[:], kv[0:1, 0:3], AF.Sin, bias=zb1[:],
                             scale=0.5)
        nc.scalar.square(kvr[:], kvr[:])
        nc.vector.tensor_scalar(ck3[:], kvr[:], -2.0, 1.0,
                                op0=OP.mult, op1=OP.add)

        # scalar slot layout in vecs[1, NSC]:
        # 0:ckN 1:skN 2:ckA 3:skA 4:ckC 5:skC
        # 6:ckNckA 7:skNskA 8:ckNskA 9:skNckA
        # 10:nskNckA 11:nckNskA 12:nckN 13:nckA 14:nckC 15:nskA
        # 16:RNckN 17:RNskN 18:RCA 19:RC
        def vc(i):
            return vecs[0:1, i:i + 1]

        for t in range(3):
            nc.vector.tensor_copy(vc(2 * t), ck3[0:1, t:t + 1])
            nc.vector.tensor_copy(vc(2 * t + 1), sk3[0:1, t:t + 1])
        nc.vector.tensor_mul(vc(6), vc(0), vc(2))     # ckN*ckA
        nc.vector.tensor_mul(vc(7), vc(1), vc(3))     # skN*skA
        nc.vector.tensor_mul(vc(8), vc(0), vc(3))     # ckN*skA
        nc.vector.tensor_mul(vc(9), vc(1), vc(2))     # skN*ckA
        nc.vector.tensor_scalar_mul(vc(10), vc(9), -1.0)
        nc.vector.tensor_scalar_mul(vc(11), vc(8), -1.0)
        nc.vector.tensor_scalar_mul(vc(12), vc(0), -1.0)
        nc.vector.tensor_scalar_mul(vc(13), vc(2), -1.0)
        nc.vector.tensor_scalar_mul(vc(14), vc(4), -1.0)
        nc.vector.tensor_scalar_mul(vc(15), vc(3), -1.0)
        nc.vector.tensor_mul(vc(16), Rv[0:1, 0:1], vc(0))   # RN*ckN
        nc.vector.tensor_mul(vc(17), Rv[0:1, 0:1], vc(1))   # RN*skN
        nc.vector.tensor_copy(vc(18), Rv[0:1, 1:2])         # R_CA
        nc.vector.tensor_copy(vc(19), Rv[0:1, 2:3])         # R_C
        nc.gpsimd.partition_broadcast(Vb[:], vecs[:])

        # selector matrices for PE-based cross-partition gather/broadcast
        rowid_i = main.tile([128, 1], I32, tag="rowid_i")
        rowid = main.tile([128, 1], F32, tag="rowid")
        colid = main.tile([128, 32], I32, tag="colid")
        rowq = main.tile([128, 1], F32, tag="rowq")
        I32f = main.tile([BPC, BPC], F32, tag="I32f")
        selq = main.tile([128, 4 * BPC], F32, tag="selq")
        nc.gpsimd.iota(rowid_i[:], [[0, 1]], channel_multiplier=1)
        nc.gpsimd.iota(colid[:], [[1, BPC]], channel_multiplier=0)
        nc.vector.tensor_copy(rowid[:], rowid_i[:])
        nc.vector.tensor_scalar(I32f[0:BPC, 0:BPC], colid[0:BPC, :],
                                rowid[0:BPC, 0:1], None, op0=OP.is_equal)
        for q in range(QN):
            nc.vector.tensor_scalar(rowq[:], rowid[:], float(q * BPC), None,
                                    op0=OP.subtract)
            nc.vector.tensor_scalar(selq[:, q * BPC:(q + 1) * BPC], colid[:],
                                    rowq[:, 0:1], None, op0=OP.is_equal)
        PSg = psum.tile([BPC, 36], F32, tag="PSg")
        PSf = psum.tile([128, 9], F32, tag="PSf")
        PSp = psum.tile([BPC, 9], F32, tag="PSp")
        PSi = psum.tile([128, 3], F32, tag="PSi")

        S = {}
        for i, nm in enumerate(("ckN", "skN", "ckA", "skA", "ckC", "skC",
                                "ckNckA", "skNskA", "ckNskA", "skNckA",
                                "nskNckA", "nckNskA", "nckN", "nckA",
                                "nckC", "nskA", "RNckN", "RNskN",
                                "RCA", "RC")):
            S[nm] = Vb[:, i:i + 1]

        # trig: |alpha| < 4pi: s4=sin(a/4), c4=1-2sin^2(a/8);
        # s2=2*s4*c4, c2=1-2*s4^2; s1=2*s2*c2, c1=1-2*s2^2.
        # One chain per 128-col type block, pipelined across ACT/DVE.
        for t in range(3):
            bs = slice(t * NR, (t + 1) * NR)
            avb, cab, sab = av[:, bs], ca[:, bs], sa[:, bs]
            ts8 = tmps.tile([128, NR], F32, tag="t1")
            ts4 = tmps.tile([128, NR], F32, tag="t2")
            tq = tmps.tile([128, NR], F32, tag="t1")
            nc.scalar.activation(ts8[:], avb, AF.Sin, bias=zb128[:],
                                 scale=0.125)
            nc.scalar.activation(ts4[:], avb, AF.Sin, bias=zb128[:],
                                 scale=0.25)
            nc.scalar.square(ts8[:], ts8[:])
            nc.vector.tensor_scalar(cab, ts8[:], -2.0, 1.0,
                                    op0=OP.mult, op1=OP.add)          # c4
            nc.vector.scalar_tensor_tensor(ts8[:], ts4[:], 2.0, cab,
                                           op0=OP.mult, op1=OP.mult)  # s2
            nc.scalar.square(tq[:], ts4[:])
            nc.vector.tensor_scalar(ts4[:], tq[:], -2.0, 1.0,
                                    op0=OP.mult, op1=OP.add)          # c2
            nc.vector.scalar_tensor_tensor(sab, ts8[:], 2.0, ts4[:],
                                           op0=OP.mult, op1=OP.mult)  # s1
            nc.scalar.square(tq[:], ts8[:])
            nc.vector.tensor_scalar(cab, tq[:], -2.0, 1.0,
                                    op0=OP.mult, op1=OP.add)          # c1
        nc.gpsimd.memset(zeros[:], 0.0)
        nc.gpsimd.memset(zeros[:], 0.0)
        nc.gpsimd.memset(ones[:], 1.0)

        # per-type strided cos/sin views [128, 128]
        cN, sN = ca[:, 0:128], sa[:, 0:128]
        cA, sA = ca[:, 128:256], sa[:, 128:256]
        cC, sC = ca[:, 256:384], sa[:, 256:384]

        def blk(t, e, lo=0, hi=NR):
            return t[:, e * NR + lo:e * NR + hi]

        V = nc.vector
        STT = nc.vector.scalar_tensor_tensor
        TS = nc.vector.tensor_scalar

        # ---------------- pre-pass: C1 = B_N @ B_CA ----------------
        pp1 = PP[:, 0 * NR:1 * NR]
        pp2 = PP[:, 1 * NR:2 * NR]
        pp3 = PP[:, 2 * NR:3 * NR]
        pp4 = PP[:, 3 * NR:4 * NR]
        nc.gpsimd.tensor_mul(pp1, cN, cA)
        nc.gpsimd.tensor_mul(pp2, sN, sA)
        nc.gpsimd.tensor_mul(pp3, cN, sA)
        nc.gpsimd.tensor_mul(pp4, sN, cA)
        c1 = C1[:]
        TS(blk(c1, 0), cA, S["skNskA"], S["ckNckA"],
           op0=OP.mult, op1=OP.add)                       # C1_00
        TS(blk(c1, 1), cA, S["nskNckA"], S["ckNskA"],
           op0=OP.mult, op1=OP.add)                       # C1_01
        V.tensor_scalar_mul(blk(c1, 2), sA, S["skN"])     # C1_02
        x = blk(c1, 3)                                    # C1_10
        nc.scalar.mul(x, cN, S["skNckA"])
        STT(x, pp1, S["nckNskA"], x, op0=OP.mult, op1=OP.add)
        STT(x, pp2, S["skA"], x, op0=OP.mult, op1=OP.add)
        x = blk(c1, 4)                                    # C1_11
        nc.scalar.mul(x, cN, S["skNskA"])
        STT(x, pp1, S["ckNckA"], x, op0=OP.mult, op1=OP.add)
        STT(x, pp2, S["nckA"], x, op0=OP.mult, op1=OP.add)
        STT(blk(c1, 5), pp3, S["nckN"], pp4,
            op0=OP.mult, op1=OP.subtract)                 # C1_12
        x = blk(c1, 6)                                    # C1_20
        nc.scalar.mul(x, sN, S["skNckA"])
        STT(x, pp4, S["nckNskA"], x, op0=OP.mult, op1=OP.add)
        STT(x, pp3, S["nskA"], x, op0=OP.mult, op1=OP.add)
        x = blk(c1, 7)                                    # C1_21
        nc.scalar.mul(x, sN, S["skNskA"])
        STT(x, pp4, S["ckNckA"], x, op0=OP.mult, op1=OP.add)
        STT(x, pp3, S["ckA"], x, op0=OP.mult, op1=OP.add)
        STT(blk(c1, 8), pp2, S["nckN"], pp1,
            op0=OP.mult, op1=OP.add)                      # C1_22

        # residue-0 of q=0: B_N := Identity => C1 := B_CA(0)
        # (alpha_CA(0)=0 so cA=1, sA=0 there): [[ckA,skA,0],[skA,-ckA,0],
        # [0,0,-1]]
        r0s = slice(0, BPC)
        o1 = ones[r0s, 0:1]
        V.tensor_scalar_mul(c1[r0s, 0 * NR:0 * NR + 1], o1, S["ckA"][r0s])
        V.tensor_scalar_mul(c1[r0s, 1 * NR:1 * NR + 1], o1, S["skA"][r0s])
        V.memset(c1[r0s, 2 * NR:2 * NR + 1], 0.0)
        V.tensor_scalar_mul(c1[r0s, 3 * NR:3 * NR + 1], o1, S["skA"][r0s])
        V.tensor_scalar_mul(c1[r0s, 4 * NR:4 * NR + 1], o1, S["nckA"][r0s])
        V.memset(c1[r0s, 5 * NR:5 * NR + 1], 0.0)
        V.memset(c1[r0s, 6 * NR:6 * NR + 1], 0.0)
        V.memset(c1[r0s, 7 * NR:7 * NR + 1], 0.0)
        V.memset(c1[r0s, 8 * NR:8 * NR + 1], -1.0)

        # ---------------- pre-pass: Rres = C1 @ B_C -> RA ----------------
        ra = RA[:]
        for i in range(3):
            nc.gpsimd.tensor_mul(blk(QQ[:], i), blk(c1, _e(i, 1)), cC)
            nc.gpsimd.tensor_mul(blk(QQ[:], 3 + i), blk(c1, _e(i, 2)), sC)
        for i in range(3):
            q1i = blk(QQ[:], i)
            q2i = blk(QQ[:], 3 + i)
            x = blk(ra, _e(i, 0))
            nc.scalar.mul(x, blk(c1, _e(i, 0)), S["ckC"])
            STT(x, q1i, S["skC"], x, op0=OP.mult, op1=OP.add)
            STT(x, q2i, S["skC"], x, op0=OP.mult, op1=OP.add)
            x = blk(ra, _e(i, 1))
            nc.scalar.mul(x, blk(c1, _e(i, 0)), S["skC"])
            STT(x, q1i, S["nckC"], x, op0=OP.mult, op1=OP.add)
            STT(x, q2i, S["nckC"], x, op0=OP.mult, op1=OP.add)
            t1 = tmps.tile([128, NR], F32, tag="pt1")
            t2 = tmps.tile([128, NR], F32, tag="pt2")
            nc.gpsimd.tensor_mul(t1[:], blk(c1, _e(i, 1)), sC)
            nc.gpsimd.tensor_mul(t2[:], blk(c1, _e(i, 2)), cC)
            nc.gpsimd.tensor_sub(blk(ra, _e(i, 2)), t1[:], t2[:])

        # ---------------- v-vectors for atom expansion ----------------
        # v1 = t_N = RN*(ckN, skN*cN, skN*sN); v2 = RCA*C1[:,0];
        # v3 = RC*Rres[:,0]
        vm = Vm[:]
        nc.scalar.mul(blk(vm, 0), ones[:], S["RNckN"])
        nc.scalar.mul(blk(vm, 1), cN, S["RNskN"])
        nc.scalar.mul(blk(vm, 2), sN, S["RNskN"])
        for i in range(3):
            nc.scalar.mul(blk(vm, 3 + i), blk(c1, _e(i, 0)), S["RCA"])
            nc.scalar.mul(blk(vm, 6 + i), blk(ra, _e(i, 0)), S["RC"])

        # ---------------- Hillis-Steele residue scan ----------------
        # Fused step: all 9 output entries in one 3-dim AP op per k-term:
        #   out[i,j] += L[i,k] (bcast over j) * R[k,j] (bcast over i)
        # 5 logical ops per step, each split col-wise DVE/Pool.
        def ap3(base_ap, off, dims):
            return bass.AP(base_ap.tensor, base_ap.offset + off,
                           [list(base_ap.ap[0])] + [list(d) for d in dims])

        T9a = main.tile([128, 9 * NR], F32, tag="T9a")
        T9b = main.tile([128, 9 * NR], F32, tag="T9b")

        def fused_step(srcb, dstb, s, nr):
            n = nr - s
            cut = (n * 5) // 8          # DVE share of columns
            sv = srcb.rearrange("p (e j) -> p e j", e=9)
            dv = dstb.rearrange("p (e j) -> p e j", e=9)
            nc.scalar.copy(dv[:, :, 0:s], sv[:, :, 0:s])

            def L(k, c0, c1):
                return ap3(srcb, k * nr + c0,
                           [[3 * nr, 3], [0, 3], [1, c1 - c0]])

            def R(k, c0, c1):
                return ap3(srcb, 3 * k * nr + s + c0,
                           [[0, 3], [nr, 3], [1, c1 - c0]])

            def T(t, c0, c1):
                return ap3(t[:], c0, [[3 * nr, 3], [nr, 3], [1, c1 - c0]])

            def O(c0, c1):
                return ap3(dstb, s + c0, [[3 * nr, 3], [nr, 3], [1, c1 - c0]])

            for E, c0, c1 in ((nc.vector, 0, cut), (nc.gpsimd, cut, n)):
                if c1 <= c0:
                    continue
                E.tensor_mul(T(T9a, c0, c1), L(0, c0, c1), R(0, c0, c1))
                E.tensor_mul(T(T9b, c0, c1), L(1, c0, c1), R(1, c0, c1))
                E.tensor_add(T(T9a, c0, c1), T(T9a, c0, c1), T(T9b, c0, c1))
                E.tensor_mul(T(T9b, c0, c1), L(2, c0, c1), R(2, c0, c1))
                E.tensor_add(O(c0, c1), T(T9a, c0, c1), T(T9b, c0, c1))

        # pair adjacent residues: P2[r'] = Rres[2r'] @ Rres[2r'+1]
        NR2 = NR // 2
        P2A = main.tile([128, 9 * NR2], F32, tag="P2A")
        P2B = main.tile([128, 9 * NR2], F32, tag="P2B")
        pcut = (NR2 * 5) // 8
        ra_ap = RA[:]

        def PL(k, c0, c1):
            return ap3(ra_ap, k * NR + 2 * c0,
                       [[3 * NR, 3], [0, 3], [2, c1 - c0]])

        def PR(k, c0, c1):
            return ap3(ra_ap, 3 * k * NR + 1 + 2 * c0,
                       [[0, 3], [NR, 3], [2, c1 - c0]])

        def PT(t, c0, c1):
            return ap3(t[:], c0, [[3 * NR2, 3], [NR2, 3], [1, c1 - c0]])

        def PO(c0, c1):
            return ap3(P2A[:], c0, [[3 * NR2, 3], [NR2, 3], [1, c1 - c0]])

        for E, c0, c1 in ((nc.vector, 0, pcut), (nc.gpsimd, pcut, NR2)):
            E.tensor_mul(PT(T9a, c0, c1), PL(0, c0, c1), PR(0, c0, c1))
            E.tensor_mul(PT(T9b, c0, c1), PL(1, c0, c1), PR(1, c0, c1))
            E.tensor_add(PT(T9a, c0, c1), PT(T9a, c0, c1), PT(T9b, c0, c1))
            E.tensor_mul(PT(T9b, c0, c1), PL(2, c0, c1), PR(2, c0, c1))
            E.tensor_add(PO(c0, c1), PT(T9a, c0, c1), PT(T9b, c0, c1))

        Wodd = main.tile([128, 9 * NR2], F32, tag="Wodd")
        T9c = main.tile([128, 9 * NR2], F32, tag="T9c")
        T9d = main.tile([128, 9 * NR2], F32, tag="T9d")
        wo = Wodd[:]
        wcut = (NR2 * 5) // 8

        def WL(k, c0, c1):
            return ap3(ra_ap, k * NR + 2 * c0,
                       [[3 * NR, 3], [0, 3], [2, c1 - c0]])

        def WR(k, c0, c1):
            return ap3(vm, k * NR + 1 + 2 * c0,
                       [[0, 3], [3 * NR, 3], [2, c1 - c0]])

        def WT(t, c0, c1):
            return ap3(t[:], c0, [[3 * NR2, 3], [NR2, 3], [1, c1 - c0]])

        def WO(c0, c1):
            return ap3(wo, c0, [[NR2, 3], [3 * NR2, 3], [1, c1 - c0]])

        for E, c0, c1 in ((nc.vector, 0, wcut), (nc.gpsimd, wcut, NR2)):
            E.tensor_mul(WT(T9a, c0, c1), WL(0, c0, c1), WR(0, c0, c1))
            E.tensor_mul(WT(T9b, c0, c1), WL(1, c0, c1), WR(1, c0, c1))
            E.tensor_add(WT(T9a, c0, c1), WT(T9a, c0, c1), WT(T9b, c0, c1))
            E.tensor_mul(WT(T9b, c0, c1), WL(2, c0, c1), WR(2, c0, c1))
            E.tensor_add(WO(c0, c1), WT(T9a, c0, c1), WT(T9b, c0, c1))


        bufs = [P2A, P2B]
        nsteps = 6
        for step in range(nsteps):
            fused_step(bufs[step % 2][:], bufs[(step + 1) % 2][:],
                       1 << step, NR2)
        Rscan = bufs[nsteps % 2][:]    # RB: local residue prefixes

        # ---------------- cross-quarter rotation fixup ----------------
        for q in range(QN):
            nc.tensor.matmul(
                PSg[0:BPC, q * 9:(q + 1) * 9],
                selq[:, q * BPC:(q + 1) * BPC],
                Rscan[:, NR2 - 1:9 * NR2:NR2], start=True, stop=True)
        nc.vector.tensor_copy(Estack[0:BPC, 0:36], PSg[0:BPC, 0:36])
        nc.vector.tensor_copy(Fstack[0:BPC, 0:9], Estack[0:BPC, 0:9])
        mt0 = main.tile([BPC, 9], F32, tag="mt0")
        mt1 = main.tile([BPC, 9], F32, tag="mt1")
        fs = Fstack[:]
        es = Estack[:]

        def ap2(base_ap, off, dims):
            return bass.AP(base_ap.tensor, base_ap.offset + off,
                           [list(base_ap.ap[0])] + [list(d) for d in dims])

        for q in (1, 2):
            FL = lambda k: ap2(fs, (q - 1) * 9 + k, [[3, 3], [0, 3]])
            ER = lambda k: ap2(es, q * 9 + 3 * k, [[0, 3], [1, 3]])
            MT = lambda t: ap2(t[:], 0, [[3, 3], [1, 3]])
            FO = ap2(fs, q * 9, [[3, 3], [1, 3]])
            V.tensor_mul(MT(mt0), FL(0), ER(0))
            V.tensor_mul(MT(mt1), FL(1), ER(1))
            V.tensor_add(MT(mt0), MT(mt0), MT(mt1))
            V.tensor_mul(MT(mt1), FL(2), ER(2))
            V.tensor_add(FO, MT(mt0), MT(mt1))
        nc.vector.memset(Fbc[0:BPC, 0:9], 0.0)
        for e in (0, 4, 8):
            nc.vector.memset(Fbc[0:BPC, e:e + 1], 1.0)
        for q in (1, 2):
            nc.tensor.matmul(
                PSf[q * BPC:(q + 1) * BPC, 0:9], I32f[0:BPC, 0:BPC],
                Fstack[0:BPC, (q - 1) * 9:q * 9], start=True, stop=True)
        for q in (1, 2):
            nc.vector.tensor_copy(Fbc[q * BPC:(q + 1) * BPC, 0:9],
                                  PSf[q * BPC:(q + 1) * BPC, 0:9])
        nc.sync.dma_start(Fbc[3 * BPC:128, 0:9], Fstack[0:BPC, 18:27])
        # ---------------- atom translations (local frame) ----------------
        # superblock = 2 residues = 6 atoms. w_m (m=0..5): prefix-within-
        # superblock applied to t-vectors; m<3 are the per-residue v's at
        # even residues, m>=3 need Rres_even @ v_odd (fused below).
        Uloc = main.tile([128, 3 * W], F32, tag="Uloc")
        ul = Uloc[:]
        rs = Rscan
        # superblock 0: local prefix = identity -> u = w_m
        V.tensor_copy(ap3(ul, 0, [[W, 3], [1, 3]]),
                      ap3(vm, 0, [[NR, 3], [3 * NR, 3]]))
        V.tensor_copy(ap3(ul, 3, [[W, 3], [1, 3]]),
                      ap3(wo, 0, [[NR2, 3], [3 * NR2, 3]]))
        nu = NR2 - 1
        ucut = (nu * 5) // 8

        def UL(k, c0, c1):
            return ap3(rs, k * NR2 + c0, [[3 * NR2, 3], [0, 3], [1, c1 - c0]])

        def UREV(k, c0, c1):   # even residues 2r', r'=1..63
            return ap3(vm, k * NR + 2 + 2 * c0,
                       [[0, 3], [3 * NR, 3], [2, c1 - c0]])

        def UROD(k, c0, c1):   # w_{3..5} at superblocks 1..63
            return ap3(wo, k * NR2 + 1 + c0,
                       [[0, 3], [3 * NR2, 3], [1, c1 - c0]])

        def UT(t, c0, c1):
            return ap3(t[:], c0, [[3 * NR2, 3], [NR2, 3], [1, c1 - c0]])

        def UO(off, c0, c1):
            return ap3(ul, off + 6 * c0, [[W, 3], [1, 3], [6, c1 - c0]])

        for E, c0, c1 in ((nc.vector, 0, ucut), (nc.gpsimd, ucut, nu)):
            E.tensor_mul(UT(T9c, c0, c1), UL(0, c0, c1), UREV(0, c0, c1))
            E.tensor_mul(UT(T9d, c0, c1), UL(1, c0, c1), UREV(1, c0, c1))
            E.tensor_add(UT(T9c, c0, c1), UT(T9c, c0, c1), UT(T9d, c0, c1))
            E.tensor_mul(UT(T9d, c0, c1), UL(2, c0, c1), UREV(2, c0, c1))
            E.tensor_add(UO(6, c0, c1), UT(T9c, c0, c1), UT(T9d, c0, c1))
        for E, c0, c1 in ((nc.vector, 0, ucut), (nc.gpsimd, ucut, nu)):
            E.tensor_mul(UT(T9a, c0, c1), UL(0, c0, c1), UROD(0, c0, c1))
            E.tensor_mul(UT(T9b, c0, c1), UL(1, c0, c1), UROD(1, c0, c1))
            E.tensor_add(UT(T9a, c0, c1), UT(T9a, c0, c1), UT(T9b, c0, c1))
            E.tensor_mul(UT(T9b, c0, c1), UL(2, c0, c1), UROD(2, c0, c1))
            E.tensor_add(UO(9, c0, c1), UT(T9a, c0, c1), UT(T9b, c0, c1))
        # prefix-sum the LOCAL u per coordinate (frame fix applied at the
        # end by linearity: sum_j F@u = F@sum_j u)
        for c in range(3):
            uc = ul[:, c * W:(c + 1) * W]
            V.memset(uc[0:BPC, 0:1], 0.0)   # atom 0 of the whole chain
            nc.vector.tensor_tensor_scan(
                Pall[:, c * W:(c + 1) * W], uc, zeros[:], 0.0,
                op0=OP.add, op1=OP.add)

        # ---------------- cross-quarter translation fixup ----------------
        pv = Pall[:]
        for q in range(3):
            nc.tensor.matmul(
                PSp[0:BPC, q * 3:(q + 1) * 3],
                selq[:, q * BPC:(q + 1) * BPC],
                pv[:, W - 1:3 * W:W], start=True, stop=True)
        nc.vector.tensor_copy(pestage[0:BPC, 0:9], PSp[0:BPC, 0:9])
        # global pe_q = F_q @ pe_local_q (F_0 = I); Fstack block q-1 = F_q
        peg = main.tile([BPC, 9], F32, tag="peg")
        ps = pestage[:]
        nc.vector.tensor_copy(peg[0:BPC, 0:3], pestage[0:BPC, 0:3])
        for q in (1, 2):
            FL = lambda k: ap2(fs, (q - 1) * 9 + k, [[3, 3]])
            PR = lambda k: ap2(ps, q * 3 + k, [[0, 3]])
            M3 = lambda t: ap2(t[:], 0, [[1, 3]])
            PO = ap2(peg[:], q * 3, [[1, 3]])
            V.tensor_mul(M3(mt0), FL(0), PR(0))
            V.tensor_mul(M3(mt1), FL(1), PR(1))
            V.tensor_add(M3(mt0), M3(mt0), M3(mt1))
            V.tensor_mul(M3(mt1), FL(2), PR(2))
            V.tensor_add(PO, M3(mt0), M3(mt1))
        nc.vector.tensor_copy(cumst[0:BPC, 0:3], peg[0:BPC, 0:3])
        nc.vector.tensor_add(cumst[0:BPC, 3:6], cumst[0:BPC, 0:3],
                             peg[0:BPC, 3:6])
        nc.vector.tensor_add(cumst[0:BPC, 6:9], cumst[0:BPC, 3:6],
                             peg[0:BPC, 6:9])
        nc.vector.memset(Pincb[0:BPC, 0:3], 0.0)
        for q in (1, 2):
            nc.tensor.matmul(
                PSi[q * BPC:(q + 1) * BPC, 0:3], I32f[0:BPC, 0:BPC],
                cumst[0:BPC, (q - 1) * 3:q * 3], start=True, stop=True)
        for q in (1, 2):
            nc.vector.tensor_copy(Pincb[q * BPC:(q + 1) * BPC, 0:3],
                                  PSi[q * BPC:(q + 1) * BPC, 0:3])
        nc.scalar.dma_start(Pincb[3 * BPC:128, 0:3], cumst[0:BPC, 6:9])

        # ---------------- mask ----------------
        nc.gpsimd.iota(jplane_i[:], [[1, W]], channel_multiplier=0)
        nc.vector.tensor_copy(jplane[:], jplane_i[:])
        nc.vector.tensor_copy(Lf[:], Lsb[:])
        for q in range(QN):
            (nc.sync if q % 2 else nc.scalar).dma_start(
                Lbc[q * BPC:(q + 1) * BPC, 0:1], Lf[:])
        for q in range(QN):
            TS(thr[q * BPC:(q + 1) * BPC, 0:1],
               Lbc[q * BPC:(q + 1) * BPC, 0:1],
               3.0, float(q * W), op0=OP.mult, op1=OP.subtract)
        TS(maskp[:], jplane[:], thr[:, 0:1], None, op0=OP.is_lt)

        # ------------- fused frame-fix + P_inc + mask + store -------------
        for c in range(3):
            x = tmps.tile([128, W], F32, tag="t1")
            V.tensor_scalar_mul(x[:], pv[:, 0:W],
                                Fbc[:, _e(c, 0):_e(c, 0) + 1])
            STT(x[:], pv[:, W:2 * W], Fbc[:, _e(c, 1):_e(c, 1) + 1], x[:],
                op0=OP.mult, op1=OP.add)
            STT(x[:], pv[:, 2 * W:3 * W], Fbc[:, _e(c, 2):_e(c, 2) + 1], x[:],
                op0=OP.mult, op1=OP.add)
            STT(Pmall[:, c * W:(c + 1) * W], x[:],
                Pincb[:, c:c + 1], maskp[:], op0=OP.add, op1=OP.mult)
        nc.sync.dma_start(out[:], Pmall[:])


def _prep_alpha(input):
    # pure indexing: alphaN[r]=psi[r-1], alphaCA[r]=omega[r-1] (0 at r=0),
    # alphaC[r]=phi[r]; blocked (q, b, type, m).
    phi, psi, om = input[:, 0], input[:, 1], input[:, 2]
    z1 = np.zeros((input.shape[0], 1), np.float32)
    aN = np.concatenate([z1, psi[:, :-1]], axis=1)
    aCA = np.concatenate([z1, om[:, :-1]], axis=1)
    alpha = np.stack([aN, aCA, phi], axis=1)          # [B, 3, 512]
    return alpha.reshape(-1, 3, QN, NR).transpose(0, 2, 1, 3)


def _shard_alpha(alpha, i):
    sl = slice(i * BPC, (i + 1) * BPC)
    return np.ascontiguousarray(
        alpha[sl].transpose(1, 0, 2, 3).reshape(QN * BPC, 3 * NR))


def _get_nc():
    if "nc" not in _CACHE:
        _CACHE["nc"] = _build_graph()
    return _CACHE["nc"]


def kernel(input, param, angles_length, trace=False):
    input = np.ascontiguousarray(input, dtype=np.float32)
    param = np.ascontiguousarray(param, dtype=np.float32)
    angles_length = np.ascontiguousarray(angles_length, dtype=np.int32)
    nc = _get_nc()
    alpha = _prep_alpha(input)
    in_maps = []
    for i in range(NCORES):
        sl = slice(i * BPC, (i + 1) * BPC)
        in_maps.append({
            "input": _shard_alpha(alpha, i),
            "param": param,
            "angles_length": angles_length[sl],
        })
    res = run_bass_kernel_spmd(nc, in_maps, core_ids=list(range(NCORES)),
                               trace=trace)
    outs = []
    for i in range(NCORES):
        r = res.results[i]["out"]          # [(q,b), (c,j)]
        r = r.reshape(QN, BPC, 3, W)
        r = np.transpose(r, (1, 0, 3, 2)).reshape(BPC, 3 * QN * W)
        outs.append(r)
    full = np.concatenate(outs, axis=0).astype(np.float32)
    if trace:
        kernel._last_exec_ns = res.exec_time_ns
    return full


kernel._last_exec_ns = None


# revision 46
# speedup vs baseline: 1.2129x; 1.0305x over previous
"""Trainium2 Bass kernel for Angles2Backbone (NeRF chain forward).

Full inputs: input [256,3,512] f32, param [6] f32, angles_length [256] i32.
Output: [256, 4608] f32  (coords of 1536 backbone atoms x 3, masked).

Sharding: pure data parallel over batch - 32 proteins per core x 8 cores.

Per-core algorithm (v2, residue-granularity scan):
  - Layout: 128 partitions = (quarter q in 0..3)*32 + protein b. Each row
    owns 128 consecutive residues (=384 atoms) of protein b's chain.
  - Pre-pass: per-residue product Rres = B_N @ B_CA @ B_C computed from
    cos/sin planes with per-type param scalars folded in (leaf structure
    collapses most terms into tensor_scalar/scalar_tensor_tensor ops).
  - Rotation prefix over 128 residues via Hillis-Steele (7 steps) on 9
    entry planes, ping-pong buffered, DVE/Pool split.
  - Cross-quarter fixup: gather quarter-end matrices, 3-step mini-scan,
    apply incoming prefix as per-partition scalars.
  - Atom translations: u_a = R_a * (first column of atom-level prefix)
    expands from residue prefixes with precomputed v-vectors; per-row
    prefix sum via hardware tensor_tensor_scan; additive cross-quarter
    fixup + length mask fused into the final store.
"""

import sys

sys.path.insert(0, "/opt/trn_rl_repo")

import numpy as np
import concourse.bass as bass
import concourse.bacc as bacc
import concourse.mybir as mybir
from concourse import tile
from concourse.bass_utils import run_bass_kernel_spmd

F32 = mybir.dt.float32
I32 = mybir.dt.int32
AF = mybir.ActivationFunctionType
OP = mybir.AluOpType

NCORES = 8
BPC = 32          # proteins per core
L = 512           # residues per protein
QN = 4            # chain quarters per protein (partition groups)
W = 384           # atoms per quarter
NR = 128          # residues per quarter (scan length)
PI = float(np.pi)

_CACHE = {}


def _e(i, k):
    return 3 * i + k


def _build_graph():
    nc = bacc.Bacc("TRN2", target_bir_lowering=False, debug=False,
                   num_devices=NCORES)
    inp = nc.dram_tensor("input", [QN * BPC, 3 * NR], F32,
                     kind="ExternalInput").ap()
    par = nc.dram_tensor("param", [6], F32, kind="ExternalInput").ap()
    alen = nc.dram_tensor("angles_length", [BPC], I32,
                          kind="ExternalInput").ap()
    out = nc.dram_tensor("out", [QN * BPC, 3 * W], F32,
                     kind="ExternalOutput").ap()

    with tile.TileContext(nc) as tc:
        _emit(nc, tc, inp, par, alen, out)
    nc.compile()
    return nc


def _emit(nc, tc, inp, par, alen, out):
    import contextlib
    ctx = contextlib.ExitStack()
    with ctx:
        main = ctx.enter_context(tc.tile_pool(name="main", bufs=1))
        tmps = ctx.enter_context(tc.tile_pool(name="tmps", bufs=18))
        psum = ctx.enter_context(tc.tile_pool(name="psum", bufs=1,
                                              space="PSUM"))

        # ---------------- persistent tiles ----------------
        alpha = main.tile([128, W], F32, tag="alpha")
        ca = main.tile([128, W], F32, tag="ca")
        sa = main.tile([128, W], F32, tag="sa")
        C1 = main.tile([128, 9 * NR], F32, tag="C1")
        RA = main.tile([128, 9 * NR], F32, tag="RA")   # residue mats / Pfull
        RB = main.tile([128, 9 * NR], F32, tag="RB")   # ping-pong
        PP = main.tile([128, 4 * NR], F32, tag="PP")   # pp1..pp4
        QQ = main.tile([128, 6 * NR], F32, tag="QQ")   # q1_i, q2_i
        Vm = main.tile([128, 9 * NR], F32, tag="Vm")   # v1,v2,v3 x 3 coords
        zeros = main.tile([128, W], F32, tag="zeros")
        ones = main.tile([128, NR], F32, tag="ones")
        Pall = main.tile([128, 3 * W], F32, tag="Pall")
        Pmall = main.tile([128, 3 * W], F32, tag="Pmall")
        jplane_i = main.tile([128, W], I32, tag="jplane_i")
        jplane = main.tile([128, W], F32, tag="jplane")
        maskp = main.tile([128, W], F32, tag="maskp")
        thr = main.tile([128, 1], F32, tag="thr")
        Lbc = main.tile([128, 1], F32, tag="Lbc")
        Lsb = main.tile([BPC, 1], I32, tag="Lsb")
        Lf = main.tile([BPC, 1], F32, tag="Lf")
        Psb = main.tile([1, 6], F32, tag="Psb")
        kv = main.tile([1, 3], F32, tag="kv")
        Rv = main.tile([1, 3], F32, tag="Rv")
        NSC = 24
        vecs = main.tile([1, NSC], F32, tag="vecs")
        Vb = main.tile([128, NSC], F32, tag="Vb")
        Estack = main.tile([BPC, 36], F32, tag="Estack")
        Fstack = main.tile([BPC, 27], F32, tag="Fstack")
        Fbc = main.tile([128, 9], F32, tag="Fbc")
        pestage = main.tile([BPC, 9], F32, tag="pestage")
        cumst = main.tile([BPC, 9], F32, tag="cumst")
        Pincb = main.tile([128, 3], F32, tag="Pincb")
        zb1 = main.tile([1, 1], F32, tag="zb1")
        zb128 = main.tile([128, 1], F32, tag="zb128")

        _cnt = [0]

        def ENG():
            # TT ops only: alternate DVE (2/3) and Pool (1/3)
            _cnt[0] += 1
            return nc.gpsimd if (_cnt[0] % 3 == 0) else nc.vector

        # ---------------- input DMA: assemble alpha ----------------
        # inp viewed as [c][q][b][m] so one DMA covers all 4 quarters
        # (dst partition order is (q, b), matching the AP walk order).
        av = alpha[:]
        nc.scalar.dma_start(Psb[:], par[:])
        nc.scalar.dma_start(Lsb[:], alen[:])
        nc.sync.dma_start(av[:, :], inp[:])

        # ---------------- param scalars ----------------
        for t, idx in enumerate((5, 1, 3)):   # kappa: CA_C_N, C_N_CA, N_CA_C
            nc.vector.tensor_copy(kv[0:1, t:t + 1], Psb[0:1, idx:idx + 1])
        for t, idx in enumerate((4, 0, 2)):   # R: R_C_N, R_N_CA, R_CA_C
            nc.vector.tensor_copy(Rv[0:1, t:t + 1], Psb[0:1, idx:idx + 1])
        nc.vector.memset(zb1[:], 0.0)
        nc.vector.memset(zb128[:], 0.0)
        # per-type ck/sk: sk=sin(kappa) (kappa in (0,pi)); ck=1-2sin^2(k/2)
        sk3 = main.tile([1, 3], F32, tag="sk3")
        ck3 = main.tile([1, 3], F32, tag="ck3")
        kvr = main.tile([1, 3], F32, tag="kvr")
        nc.scalar.activation(sk3[:], kv[0:1, 0:3], AF.Sin, bias=zb1[:])
        nc.scalar.activation(kvr[:], kv[0:1, 0:3], AF.Sin, bias=zb1[:],
                             scale=0.5)
        nc.scalar.square(kvr[:], kvr[:])
        nc.vector.tensor_scalar(ck3[:], kvr[:], -2.0, 1.0,
                                op0=OP.mult, op1=OP.add)

        # scalar slot layout in vecs[1, NSC]:
        # 0:ckN 1:skN 2:ckA 3:skA 4:ckC 5:skC
        # 6:ckNckA 7:skNskA 8:ckNskA 9:skNckA
        # 10:nskNckA 11:nckNskA 12:nckN 13:nckA 14:nckC 15:nskA
        # 16:RNckN 17:RNskN 18:RCA 19:RC
        def vc(i):
            return vecs[0:1, i:i + 1]

        for t in range(3):
            nc.vector.tensor_copy(vc(2 * t), ck3[0:1, t:t + 1])
            nc.vector.tensor_copy(vc(2 * t + 1), sk3[0:1, t:t + 1])
        nc.vector.tensor_mul(vc(6), vc(0), vc(2))     # ckN*ckA
        nc.vector.tensor_mul(vc(7), vc(1), vc(3))     # skN*skA
        nc.vector.tensor_mul(vc(8), vc(0), vc(3))     # ckN*skA
        nc.vector.tensor_mul(vc(9), vc(1), vc(2))     # skN*ckA
        nc.vector.tensor_scalar_mul(vc(10), vc(9), -1.0)
        nc.vector.tensor_scalar_mul(vc(11), vc(8), -1.0)
        nc.vector.tensor_scalar_mul(vc(12), vc(0), -1.0)
        nc.vector.tensor_scalar_mul(vc(13), vc(2), -1.0)
        nc.vector.tensor_scalar_mul(vc(14), vc(4), -1.0)
        nc.vector.tensor_scalar_mul(vc(15), vc(3), -1.0)
        nc.vector.tensor_mul(vc(16), Rv[0:1, 0:1], vc(0))   # RN*ckN
        nc.vector.tensor_mul(vc(17), Rv[0:1, 0:1], vc(1))   # RN*skN
        nc.vector.tensor_copy(vc(18), Rv[0:1, 1:2])         # R_CA
        nc.vector.tensor_copy(vc(19), Rv[0:1, 2:3])         # R_C
        nc.gpsimd.partition_broadcast(Vb[:], vecs[:])

        # selector matrices for PE-based cross-partition gather/broadcast
        rowid_i = main.tile([128, 1], I32, tag="rowid_i")
        rowid = main.tile([128, 1], F32, tag="rowid")
        colid = main.tile([128, 32], I32, tag="colid")
        rowq = main.tile([128, 1], F32, tag="rowq")
        I32f = main.tile([BPC, BPC], F32, tag="I32f")
        selq = main.tile([128, 4 * BPC], F32, tag="selq")
        nc.gpsimd.iota(rowid_i[:], [[0, 1]], channel_multiplier=1)
        nc.gpsimd.iota(colid[:], [[1, BPC]], channel_multiplier=0)
        nc.vector.tensor_copy(rowid[:], rowid_i[:])
        nc.vector.tensor_scalar(I32f[0:BPC, 0:BPC], colid[0:BPC, :],
                                rowid[0:BPC, 0:1], None, op0=OP.is_equal)
        for q in range(QN):
            nc.vector.tensor_scalar(rowq[:], rowid[:], float(q * BPC), None,
                                    op0=OP.subtract)
            nc.vector.tensor_scalar(selq[:, q * BPC:(q + 1) * BPC], colid[:],
                                    rowq[:, 0:1], None, op0=OP.is_equal)
        PSg = psum.tile([BPC, 36], F32, tag="PSg")
        PSf = psum.tile([128, 9], F32, tag="PSf")
        PSp = psum.tile([BPC, 9], F32, tag="PSp")
        PSi = psum.tile([128, 3], F32, tag="PSi")

        S = {}
        for i, nm in enumerate(("ckN", "skN", "ckA", "skA", "ckC", "skC",
                                "ckNckA", "skNskA", "ckNskA", "skNckA",
                                "nskNckA", "nckNskA", "nckN", "nckA",
                                "nckC", "nskA", "RNckN", "RNskN",
                                "RCA", "RC")):
            S[nm] = Vb[:, i:i + 1]

        # trig: |alpha| < 4pi: s4=sin(a/4), c4=1-2sin^2(a/8);
        # s2=2*s4*c4, c2=1-2*s4^2; s1=2*s2*c2, c1=1-2*s2^2.
        # One chain per 128-col type block, pipelined across ACT/DVE.
        for t in range(3):
            bs = slice(t * NR, (t + 1) * NR)
            avb, cab, sab = av[:, bs], ca[:, bs], sa[:, bs]
            ts8 = tmps.tile([128, NR], F32, tag="t1")
            ts4 = tmps.tile([128, NR], F32, tag="t2")
            tq = tmps.tile([128, NR], F32, tag="t1")
            nc.scalar.activation(ts8[:], avb, AF.Sin, bias=zb128[:],
                                 scale=0.125)
            nc.scalar.activation(ts4[:], avb, AF.Sin, bias=zb128[:],
                                 scale=0.25)
            nc.scalar.square(ts8[:], ts8[:])
            nc.vector.tensor_scalar(cab, ts8[:], -2.0, 1.0,
                                    op0=OP.mult, op1=OP.add)          # c4
            nc.vector.scalar_tensor_tensor(ts8[:], ts4[:], 2.0, cab,
                                           op0=OP.mult, op1=OP.mult)  # s2
            nc.scalar.square(tq[:], ts4[:])
            nc.vector.tensor_scalar(ts4[:], tq[:], -2.0, 1.0,
                                    op0=OP.mult, op1=OP.add)          # c2
            nc.vector.scalar_tensor_tensor(sab, ts8[:], 2.0, ts4[:],
                                           op0=OP.mult, op1=OP.mult)  # s1
            nc.scalar.square(tq[:], ts8[:])
            nc.vector.tensor_scalar(cab, tq[:], -2.0, 1.0,
                                    op0=OP.mult, op1=OP.add)          # c1
        nc.gpsimd.memset(zeros[:], 0.0)
        nc.gpsimd.memset(zeros[:], 0.0)
        nc.gpsimd.memset(ones[:], 1.0)

        # per-type strided cos/sin views [128, 128]
        cN, sN = ca[:, 0:128], sa[:, 0:128]
        cA, sA = ca[:, 128:256], sa[:, 128:256]
        cC, sC = ca[:, 256:384], sa[:, 256:384]

        def blk(t, e, lo=0, hi=NR):
            return t[:, e * NR + lo:e * NR + hi]

        V = nc.vector
        STT = nc.vector.scalar_tensor_tensor
        TS = nc.vector.tensor_scalar

        # ---------------- pre-pass: C1 = B_N @ B_CA ----------------
        pp1 = PP[:, 0 * NR:1 * NR]
        pp2 = PP[:, 1 * NR:2 * NR]
        pp3 = PP[:, 2 * NR:3 * NR]
        pp4 = PP[:, 3 * NR:4 * NR]
        nc.gpsimd.tensor_mul(pp1, cN, cA)
        nc.gpsimd.tensor_mul(pp2, sN, sA)
        nc.gpsimd.tensor_mul(pp3, cN, sA)
        nc.gpsimd.tensor_mul(pp4, sN, cA)
        c1 = C1[:]
        TS(blk(c1, 0), cA, S["skNskA"], S["ckNckA"],
           op0=OP.mult, op1=OP.add)                       # C1_00
        TS(blk(c1, 1), cA, S["nskNckA"], S["ckNskA"],
           op0=OP.mult, op1=OP.add)                       # C1_01
        V.tensor_scalar_mul(blk(c1, 2), sA, S["skN"])     # C1_02
        x = blk(c1, 3)                                    # C1_10
        nc.scalar.mul(x, cN, S["skNckA"])
        STT(x, pp1, S["nckNskA"], x, op0=OP.mult, op1=OP.add)
        STT(x, pp2, S["skA"], x, op0=OP.mult, op1=OP.add)
        x = blk(c1, 4)                                    # C1_11
        nc.scalar.mul(x, cN, S["skNskA"])
        STT(x, pp1, S["ckNckA"], x, op0=OP.mult, op1=OP.add)
        STT(x, pp2, S["nckA"], x, op0=OP.mult, op1=OP.add)
        STT(blk(c1, 5), pp3, S["nckN"], pp4,
            op0=OP.mult, op1=OP.subtract)                 # C1_12
        x = blk(c1, 6)                                    # C1_20
        nc.scalar.mul(x, sN, S["skNckA"])
        STT(x, pp4, S["nckNskA"], x, op0=OP.mult, op1=OP.add)
        STT(x, pp3, S["nskA"], x, op0=OP.mult, op1=OP.add)
        x = blk(c1, 7)                                    # C1_21
        nc.scalar.mul(x, sN, S["skNskA"])
        STT(x, pp4, S["ckNckA"], x, op0=OP.mult, op1=OP.add)
        STT(x, pp3, S["ckA"], x, op0=OP.mult, op1=OP.add)
        STT(blk(c1, 8), pp2, S["nckN"], pp1,
            op0=OP.mult, op1=OP.add)                      # C1_22

        # residue-0 of q=0: B_N := Identity => C1 := B_CA(0)
        # (alpha_CA(0)=0 so cA=1, sA=0 there): [[ckA,skA,0],[skA,-ckA,0],
        # [0,0,-1]]
        r0s = slice(0, BPC)
        o1 = ones[r0s, 0:1]
        V.tensor_scalar_mul(c1[r0s, 0 * NR:0 * NR + 1], o1, S["ckA"][r0s])
        V.tensor_scalar_mul(c1[r0s, 1 * NR:1 * NR + 1], o1, S["skA"][r0s])
        V.memset(c1[r0s, 2 * NR:2 * NR + 1], 0.0)
        V.tensor_scalar_mul(c1[r0s, 3 * NR:3 * NR + 1], o1, S["skA"][r0s])
        V.tensor_scalar_mul(c1[r0s, 4 * NR:4 * NR + 1], o1, S["nckA"][r0s])
        V.memset(c1[r0s, 5 * NR:5 * NR + 1], 0.0)
        V.memset(c1[r0s, 6 * NR:6 * NR + 1], 0.0)
        V.memset(c1[r0s, 7 * NR:7 * NR + 1], 0.0)
        V.memset(c1[r0s, 8 * NR:8 * NR + 1], -1.0)

        # ---------------- pre-pass: Rres = C1 @ B_C -> RA ----------------
        ra = RA[:]
        for i in range(3):
            nc.gpsimd.tensor_mul(blk(QQ[:], i), blk(c1, _e(i, 1)), cC)
            nc.gpsimd.tensor_mul(blk(QQ[:], 3 + i), blk(c1, _e(i, 2)), sC)
        for i in range(3):
            q1i = blk(QQ[:], i)
            q2i = blk(QQ[:], 3 + i)
            x = blk(ra, _e(i, 0))
            nc.scalar.mul(x, blk(c1, _e(i, 0)), S["ckC"])
            STT(x, q1i, S["skC"], x, op0=OP.mult, op1=OP.add)
            STT(x, q2i, S["skC"], x, op0=OP.mult, op1=OP.add)
            x = blk(ra, _e(i, 1))
            nc.scalar.mul(x, blk(c1, _e(i, 0)), S["skC"])
            STT(x, q1i, S["nckC"], x, op0=OP.mult, op1=OP.add)
            STT(x, q2i, S["nckC"], x, op0=OP.mult, op1=OP.add)
            t1 = tmps.tile([128, NR], F32, tag="pt1")
            t2 = tmps.tile([128, NR], F32, tag="pt2")
            nc.gpsimd.tensor_mul(t1[:], blk(c1, _e(i, 1)), sC)
            nc.gpsimd.tensor_mul(t2[:], blk(c1, _e(i, 2)), cC)
            nc.gpsimd.tensor_sub(blk(ra, _e(i, 2)), t1[:], t2[:])

        # ---------------- v-vectors for atom expansion ----------------
        # v1 = t_N = RN*(ckN, skN*cN, skN*sN); v2 = RCA*C1[:,0];
        # v3 = RC*Rres[:,0]
        vm = Vm[:]
        nc.scalar.mul(blk(vm, 0), ones[:], S["RNckN"])
        nc.scalar.mul(blk(vm, 1), cN, S["RNskN"])
        nc.scalar.mul(blk(vm, 2), sN, S["RNskN"])
        for i in range(3):
            nc.scalar.mul(blk(vm, 3 + i), blk(c1, _e(i, 0)), S["RCA"])
            nc.scalar.mul(blk(vm, 6 + i), blk(ra, _e(i, 0)), S["RC"])

        # ---------------- Hillis-Steele residue scan ----------------
        # Fused step: all 9 output entries in one 3-dim AP op per k-term:
        #   out[i,j] += L[i,k] (bcast over j) * R[k,j] (bcast over i)
        # 5 logical ops per step, each split col-wise DVE/Pool.
        def ap3(base_ap, off, dims):
            return bass.AP(base_ap.tensor, base_ap.offset + off,
                           [list(base_ap.ap[0])] + [list(d) for d in dims])

        T9a = main.tile([128, 9 * NR], F32, tag="T9a")
        T9b = main.tile([128, 9 * NR], F32, tag="T9b")

        def fused_step(srcb, dstb, s, nr):
            n = nr - s
            cut = (n * 5) // 8          # DVE share of columns
            sv = srcb.rearrange("p (e j) -> p e j", e=9)
            dv = dstb.rearrange("p (e j) -> p e j", e=9)
            nc.scalar.copy(dv[:, :, 0:s], sv[:, :, 0:s])

            def L(k, c0, c1):
                return ap3(srcb, k * nr + c0,
                           [[3 * nr, 3], [0, 3], [1, c1 - c0]])

            def R(k, c0, c1):
                return ap3(srcb, 3 * k * nr + s + c0,
                           [[0, 3], [nr, 3], [1, c1 - c0]])

            def T(t, c0, c1):
                return ap3(t[:], c0, [[3 * nr, 3], [nr, 3], [1, c1 - c0]])

            def O(c0, c1):
                return ap3(dstb, s + c0, [[3 * nr, 3], [nr, 3], [1, c1 - c0]])

            for E, c0, c1 in ((nc.vector, 0, cut), (nc.gpsimd, cut, n)):
                if c1 <= c0:
                    continue
                E.tensor_mul(T(T9a, c0, c1), L(0, c0, c1), R(0, c0, c1))
                E.tensor_mul(T(T9b, c0, c1), L(1, c0, c1), R(1, c0, c1))
                E.tensor_add(T(T9a, c0, c1), T(T9a, c0, c1), T(T9b, c0, c1))
                E.tensor_mul(T(T9b, c0, c1), L(2, c0, c1), R(2, c0, c1))
                E.tensor_add(O(c0, c1), T(T9a, c0, c1), T(T9b, c0, c1))

        # pair adjacent residues: P2[r'] = Rres[2r'] @ Rres[2r'+1]
        NR2 = NR // 2
        P2A = main.tile([128, 9 * NR2], F32, tag="P2A")
        P2B = main.tile([128, 9 * NR2], F32, tag="P2B")
        pcut = (NR2 * 5) // 8
        ra_ap = RA[:]

        def PL(k, c0, c1):
            return ap3(ra_ap, k * NR + 2 * c0,
                       [[3 * NR, 3], [0, 3], [2, c1 - c0]])

        def PR(k, c0, c1):
            return ap3(ra_ap, 3 * k * NR + 1 + 2 * c0,
                       [[0, 3], [NR, 3], [2, c1 - c0]])

        def PT(t, c0, c1):
            return ap3(t[:], c0, [[3 * NR2, 3], [NR2, 3], [1, c1 - c0]])

        def PO(c0, c1):
            return ap3(P2A[:], c0, [[3 * NR2, 3], [NR2, 3], [1, c1 - c0]])

        for E, c0, c1 in ((nc.vector, 0, pcut), (nc.gpsimd, pcut, NR2)):
            E.tensor_mul(PT(T9a, c0, c1), PL(0, c0, c1), PR(0, c0, c1))
            E.tensor_mul(PT(T9b, c0, c1), PL(1, c0, c1), PR(1, c0, c1))
            E.tensor_add(PT(T9a, c0, c1), PT(T9a, c0, c1), PT(T9b, c0, c1))
            E.tensor_mul(PT(T9b, c0, c1), PL(2, c0, c1), PR(2, c0, c1))
            E.tensor_add(PO(c0, c1), PT(T9a, c0, c1), PT(T9b, c0, c1))

        Wodd = main.tile([128, 9 * NR2], F32, tag="Wodd")
        T9c = main.tile([128, 9 * NR2], F32, tag="T9c")
        T9d = main.tile([128, 9 * NR2], F32, tag="T9d")
        wo = Wodd[:]
        wcut = (NR2 * 5) // 8

        def WL(k, c0, c1):
            return ap3(ra_ap, k * NR + 2 * c0,
                       [[3 * NR, 3], [0, 3], [2, c1 - c0]])

        def WR(k, c0, c1):
            return ap3(vm, k * NR + 1 + 2 * c0,
                       [[0, 3], [3 * NR, 3], [2, c1 - c0]])

        def WT(t, c0, c1):
            return ap3(t[:], c0, [[3 * NR2, 3], [NR2, 3], [1, c1 - c0]])

        def WO(c0, c1):
            return ap3(wo, c0, [[NR2, 3], [3 * NR2, 3], [1, c1 - c0]])

        for E, c0, c1 in ((nc.vector, 0, wcut), (nc.gpsimd, wcut, NR2)):
            E.tensor_mul(WT(T9a, c0, c1), WL(0, c0, c1), WR(0, c0, c1))
            E.tensor_mul(WT(T9b, c0, c1), WL(1, c0, c1), WR(1, c0, c1))
            E.tensor_add(WT(T9a, c0, c1), WT(T9a, c0, c1), WT(T9b, c0, c1))
            E.tensor_mul(WT(T9b, c0, c1), WL(2, c0, c1), WR(2, c0, c1))
            E.tensor_add(WO(c0, c1), WT(T9a, c0, c1), WT(T9b, c0, c1))


        # quad level: P4[r''] = P2[2r''] @ P2[2r''+1]
        NR4 = NR2 // 2
        P4A = main.tile([128, 9 * NR4], F32, tag="P4A")
        P4B = main.tile([128, 9 * NR4], F32, tag="P4B")
        W2 = main.tile([128, 18 * NR4], F32, tag="W2")
        p2a = P2A[:]
        w2 = W2[:]
        qcut = (NR4 * 5) // 8

        def QL(k, c0, c1):
            return ap3(p2a, k * NR2 + 2 * c0,
                       [[3 * NR2, 3], [0, 3], [2, c1 - c0]])

        def QR(k, c0, c1):
            return ap3(p2a, 3 * k * NR2 + 1 + 2 * c0,
                       [[0, 3], [NR2, 3], [2, c1 - c0]])

        def QT(t, c0, c1):
            return ap3(t[:], c0, [[3 * NR4, 3], [NR4, 3], [1, c1 - c0]])

        def QO(c0, c1):
            return ap3(P4A[:], c0, [[3 * NR4, 3], [NR4, 3], [1, c1 - c0]])

        for E, c0, c1 in ((nc.vector, 0, qcut), (nc.gpsimd, qcut, NR4)):
            E.tensor_mul(QT(T9a, c0, c1), QL(0, c0, c1), QR(0, c0, c1))
            E.tensor_mul(QT(T9b, c0, c1), QL(1, c0, c1), QR(1, c0, c1))
            E.tensor_add(QT(T9a, c0, c1), QT(T9a, c0, c1), QT(T9b, c0, c1))
            E.tensor_mul(QT(T9b, c0, c1), QL(2, c0, c1), QR(2, c0, c1))
            E.tensor_add(QO(c0, c1), QT(T9a, c0, c1), QT(T9b, c0, c1))

        # W2 group A (m~=0..2): P2_even @ (vm at residues 4r''+2)
        # W2 group B (m~=3..5): P2_even @ (Wodd at odd superblocks)
        def W2L(k, c0, c1):
            return ap3(p2a, k * NR2 + 2 * c0,
                       [[3 * NR2, 3], [0, 3], [2, c1 - c0]])

        def W2RA(k, c0, c1):
            return ap3(vm, k * NR + 2 + 4 * c0,
                       [[0, 3], [3 * NR, 3], [4, c1 - c0]])

        def W2RB(k, c0, c1):
            return ap3(wo, k * NR2 + 1 + 2 * c0,
                       [[0, 3], [3 * NR2, 3], [2, c1 - c0]])

        def W2O(goff, c0, c1):
            return ap3(w2, goff + c0, [[NR4, 3], [3 * NR4, 3], [1, c1 - c0]])

        for goff, RF in ((0, W2RA), (9 * NR4, W2RB)):
            for E, c0, c1 in ((nc.vector, 0, qcut), (nc.gpsimd, qcut, NR4)):
                E.tensor_mul(QT(T9c, c0, c1), W2L(0, c0, c1), RF(0, c0, c1))
                E.tensor_mul(QT(T9d, c0, c1), W2L(1, c0, c1), RF(1, c0, c1))
                E.tensor_add(QT(T9c, c0, c1), QT(T9c, c0, c1),
                             QT(T9d, c0, c1))
                E.tensor_mul(QT(T9d, c0, c1), W2L(2, c0, c1), RF(2, c0, c1))
                E.tensor_add(W2O(goff, c0, c1), QT(T9c, c0, c1),
                             QT(T9d, c0, c1))

        bufs = [P4A, P4B]
        nsteps = 5
        for step in range(nsteps):
            fused_step(bufs[step % 2][:], bufs[(step + 1) % 2][:],
                       1 << step, NR4)
        Rscan = bufs[nsteps % 2][:]    # RB: local residue prefixes

        # ---------------- cross-quarter rotation fixup ----------------
        for q in range(QN):
            nc.tensor.matmul(
                PSg[0:BPC, q * 9:(q + 1) * 9],
                selq[:, q * BPC:(q + 1) * BPC],
                Rscan[:, NR4 - 1:9 * NR4:NR4], start=True, stop=True)
        nc.vector.tensor_copy(Estack[0:BPC, 0:36], PSg[0:BPC, 0:36])
        nc.vector.tensor_copy(Fstack[0:BPC, 0:9], Estack[0:BPC, 0:9])
        mt0 = main.tile([BPC, 9], F32, tag="mt0")
        mt1 = main.tile([BPC, 9], F32, tag="mt1")
        fs = Fstack[:]
        es = Estack[:]

        def ap2(base_ap, off, dims):
            return bass.AP(base_ap.tensor, base_ap.offset + off,
                           [list(base_ap.ap[0])] + [list(d) for d in dims])

        for q in (1, 2):
            FL = lambda k: ap2(fs, (q - 1) * 9 + k, [[3, 3], [0, 3]])
            ER = lambda k: ap2(es, q * 9 + 3 * k, [[0, 3], [1, 3]])
            MT = lambda t: ap2(t[:], 0, [[3, 3], [1, 3]])
            FO = ap2(fs, q * 9, [[3, 3], [1, 3]])
            V.tensor_mul(MT(mt0), FL(0), ER(0))
            V.tensor_mul(MT(mt1), FL(1), ER(1))
            V.tensor_add(MT(mt0), MT(mt0), MT(mt1))
            V.tensor_mul(MT(mt1), FL(2), ER(2))
            V.tensor_add(FO, MT(mt0), MT(mt1))
        nc.vector.memset(Fbc[0:BPC, 0:9], 0.0)
        for e in (0, 4, 8):
            nc.vector.memset(Fbc[0:BPC, e:e + 1], 1.0)
        for q in (1, 2):
            nc.tensor.matmul(
                PSf[q * BPC:(q + 1) * BPC, 0:9], I32f[0:BPC, 0:BPC],
                Fstack[0:BPC, (q - 1) * 9:q * 9], start=True, stop=True)
        for q in (1, 2):
            nc.vector.tensor_copy(Fbc[q * BPC:(q + 1) * BPC, 0:9],
                                  PSf[q * BPC:(q + 1) * BPC, 0:9])
        nc.sync.dma_start(Fbc[3 * BPC:128, 0:9], Fstack[0:BPC, 18:27])
        # ---------------- atom translations (local frame) ----------------
        # superblock = 2 residues = 6 atoms. w_m (m=0..5): prefix-within-
        # superblock applied to t-vectors; m<3 are the per-residue v's at
        # even residues, m>=3 need Rres_even @ v_odd (fused below).
        Uloc = main.tile([128, 3 * W], F32, tag="Uloc")
        ul = Uloc[:]
        rs = Rscan
        # superblock 0: local prefix = identity -> u = w_m
        V.tensor_copy(ap3(ul, 0, [[W, 3], [1, 3]]),
                      ap3(vm, 0, [[NR, 3], [3 * NR, 3]]))
        V.tensor_copy(ap3(ul, 3, [[W, 3], [1, 3]]),
                      ap3(wo, 0, [[NR2, 3], [3 * NR2, 3]]))
        # superblock-0 atoms 6..11 from W2 (local prefix = identity)
        V.tensor_copy(ap3(ul, 6, [[W, 3], [1, 3]]),
                      ap3(w2, 0, [[NR4, 3], [3 * NR4, 3]]))
        V.tensor_copy(ap3(ul, 9, [[W, 3], [1, 3]]),
                      ap3(w2, 9 * NR4, [[NR4, 3], [3 * NR4, 3]]))
        nu = NR4 - 1
        ucut = (nu * 5) // 8

        def UL(k, c0, c1):
            return ap3(rs, k * NR4 + c0, [[3 * NR4, 3], [0, 3], [1, c1 - c0]])

        def UR0(k, c0, c1):   # atoms 12r''+0..2: vm at residue 4r''
            return ap3(vm, k * NR + 4 + 4 * c0,
                       [[0, 3], [3 * NR, 3], [4, c1 - c0]])

        def UR1(k, c0, c1):   # atoms +3..5: Wodd at superblock 2r''
            return ap3(wo, k * NR2 + 2 + 2 * c0,
                       [[0, 3], [3 * NR2, 3], [2, c1 - c0]])

        def UR2(k, c0, c1):   # atoms +6..8: W2 group A
            return ap3(w2, k * NR4 + 1 + c0,
                       [[0, 3], [3 * NR4, 3], [1, c1 - c0]])

        def UR3(k, c0, c1):   # atoms +9..11: W2 group B
            return ap3(w2, 9 * NR4 + k * NR4 + 1 + c0,
                       [[0, 3], [3 * NR4, 3], [1, c1 - c0]])

        def UT(t, c0, c1):
            return ap3(t[:], c0, [[3 * NR4, 3], [NR4, 3], [1, c1 - c0]])

        def UO(off, c0, c1):
            return ap3(ul, off + 12 * c0, [[W, 3], [1, 3], [12, c1 - c0]])

        for gi, (off, RF) in enumerate(((12, UR0), (15, UR1),
                                        (18, UR2), (21, UR3))):
            ta = [T9a, T9c][gi % 2]
            tb = [T9b, T9d][gi % 2]
            for E, c0, c1 in ((nc.vector, 0, ucut), (nc.gpsimd, ucut, nu)):
                E.tensor_mul(UT(ta, c0, c1), UL(0, c0, c1), RF(0, c0, c1))
                E.tensor_mul(UT(tb, c0, c1), UL(1, c0, c1), RF(1, c0, c1))
                E.tensor_add(UT(ta, c0, c1), UT(ta, c0, c1), UT(tb, c0, c1))
                E.tensor_mul(UT(tb, c0, c1), UL(2, c0, c1), RF(2, c0, c1))
                E.tensor_add(UO(off, c0, c1), UT(ta, c0, c1), UT(tb, c0, c1))
        # prefix-sum the LOCAL u per coordinate (frame fix applied at the
        # end by linearity: sum_j F@u = F@sum_j u)
        for c in range(3):
            uc = ul[:, c * W:(c + 1) * W]
            V.memset(uc[0:BPC, 0:1], 0.0)   # atom 0 of the whole chain
            nc.vector.tensor_tensor_scan(
                Pall[:, c * W:(c + 1) * W], uc, zeros[:], 0.0,
                op0=OP.add, op1=OP.add)

        # ---------------- cross-quarter translation fixup ----------------
        pv = Pall[:]
        for q in range(3):
            nc.tensor.matmul(
                PSp[0:BPC, q * 3:(q + 1) * 3],
                selq[:, q * BPC:(q + 1) * BPC],
                pv[:, W - 1:3 * W:W], start=True, stop=True)
        nc.vector.tensor_copy(pestage[0:BPC, 0:9], PSp[0:BPC, 0:9])
        # global pe_q = F_q @ pe_local_q (F_0 = I); Fstack block q-1 = F_q
        peg = main.tile([BPC, 9], F32, tag="peg")
        ps = pestage[:]
        nc.vector.tensor_copy(peg[0:BPC, 0:3], pestage[0:BPC, 0:3])
        for q in (1, 2):
            FL = lambda k: ap2(fs, (q - 1) * 9 + k, [[3, 3]])
            PR = lambda k: ap2(ps, q * 3 + k, [[0, 3]])
            M3 = lambda t: ap2(t[:], 0, [[1, 3]])
            PO = ap2(peg[:], q * 3, [[1, 3]])
            V.tensor_mul(M3(mt0), FL(0), PR(0))
            V.tensor_mul(M3(mt1), FL(1), PR(1))
            V.tensor_add(M3(mt0), M3(mt0), M3(mt1))
            V.tensor_mul(M3(mt1), FL(2), PR(2))
            V.tensor_add(PO, M3(mt0), M3(mt1))
        nc.vector.tensor_copy(cumst[0:BPC, 0:3], peg[0:BPC, 0:3])
        nc.vector.tensor_add(cumst[0:BPC, 3:6], cumst[0:BPC, 0:3],
                             peg[0:BPC, 3:6])
        nc.vector.tensor_add(cumst[0:BPC, 6:9], cumst[0:BPC, 3:6],
                             peg[0:BPC, 6:9])
        nc.vector.memset(Pincb[0:BPC, 0:3], 0.0)
        for q in (1, 2):
            nc.tensor.matmul(
                PSi[q * BPC:(q + 1) * BPC, 0:3], I32f[0:BPC, 0:BPC],
                cumst[0:BPC, (q - 1) * 3:q * 3], start=True, stop=True)
        for q in (1, 2):
            nc.vector.tensor_copy(Pincb[q * BPC:(q + 1) * BPC, 0:3],
                                  PSi[q * BPC:(q + 1) * BPC, 0:3])
        nc.scalar.dma_start(Pincb[3 * BPC:128, 0:3], cumst[0:BPC, 6:9])

        # ---------------- mask ----------------
        nc.gpsimd.iota(jplane_i[:], [[1, W]], channel_multiplier=0)
        nc.vector.tensor_copy(jplane[:], jplane_i[:])
        nc.vector.tensor_copy(Lf[:], Lsb[:])
        for q in range(QN):
            (nc.sync if q % 2 else nc.scalar).dma_start(
                Lbc[q * BPC:(q + 1) * BPC, 0:1], Lf[:])
        for q in range(QN):
            TS(thr[q * BPC:(q + 1) * BPC, 0:1],
               Lbc[q * BPC:(q + 1) * BPC, 0:1],
               3.0, float(q * W), op0=OP.mult, op1=OP.subtract)
        TS(maskp[:], jplane[:], thr[:, 0:1], None, op0=OP.is_lt)

        # ------------- fused frame-fix + P_inc + mask + store -------------
        for c in range(3):
            x = tmps.tile([128, W], F32, tag="t1")
            V.tensor_scalar_mul(x[:], pv[:, 0:W],
                                Fbc[:, _e(c, 0):_e(c, 0) + 1])
            STT(x[:], pv[:, W:2 * W], Fbc[:, _e(c, 1):_e(c, 1) + 1], x[:],
                op0=OP.mult, op1=OP.add)
            STT(x[:], pv[:, 2 * W:3 * W], Fbc[:, _e(c, 2):_e(c, 2) + 1], x[:],
                op0=OP.mult, op1=OP.add)
            STT(Pmall[:, c * W:(c + 1) * W], x[:],
                Pincb[:, c:c + 1], maskp[:], op0=OP.add, op1=OP.mult)
        nc.sync.dma_start(out[:], Pmall[:])


def _prep_alpha(input):
    # pure indexing: alphaN[r]=psi[r-1], alphaCA[r]=omega[r-1] (0 at r=0),
    # alphaC[r]=phi[r]; blocked (q, b, type, m).
    phi, psi, om = input[:, 0], input[:, 1], input[:, 2]
    z1 = np.zeros((input.shape[0], 1), np.float32)
    aN = np.concatenate([z1, psi[:, :-1]], axis=1)
    aCA = np.concatenate([z1, om[:, :-1]], axis=1)
    alpha = np.stack([aN, aCA, phi], axis=1)          # [B, 3, 512]
    return alpha.reshape(-1, 3, QN, NR).transpose(0, 2, 1, 3)


def _shard_alpha(alpha, i):
    sl = slice(i * BPC, (i + 1) * BPC)
    return np.ascontiguousarray(
        alpha[sl].transpose(1, 0, 2, 3).reshape(QN * BPC, 3 * NR))


def _get_nc():
    if "nc" not in _CACHE:
        _CACHE["nc"] = _build_graph()
    return _CACHE["nc"]


def kernel(input, param, angles_length, trace=False):
    input = np.ascontiguousarray(input, dtype=np.float32)
    param = np.ascontiguousarray(param, dtype=np.float32)
    angles_length = np.ascontiguousarray(angles_length, dtype=np.int32)
    nc = _get_nc()
    alpha = _prep_alpha(input)
    in_maps = []
    for i in range(NCORES):
        sl = slice(i * BPC, (i + 1) * BPC)
        in_maps.append({
            "input": _shard_alpha(alpha, i),
            "param": param,
            "angles_length": angles_length[sl],
        })
    res = run_bass_kernel_spmd(nc, in_maps, core_ids=list(range(NCORES)),
                               trace=trace)
    outs = []
    for i in range(NCORES):
        r = res.results[i]["out"]          # [(q,b), (c,j)]
        r = r.reshape(QN, BPC, 3, W)
        r = np.transpose(r, (1, 0, 3, 2)).reshape(BPC, 3 * QN * W)
        outs.append(r)
    full = np.concatenate(outs, axis=0).astype(np.float32)
    if trace:
        kernel._last_exec_ns = res.exec_time_ns
    return full


kernel._last_exec_ns = None
